# revision 1
# baseline (speedup 1.0000x reference)
"""Trainium2 Bass kernel for nn_FATMSparse (spiking Haar-wavelet network).

Sharding: the 256 channels are split 32-per-core across 8 cores. Every
stage of the network (LIF, Haar transforms, all five training-mode
BatchNorms, the per-16-channel block-diagonal mixes and both grouped
convolutions) is exactly local to an aligned 32-channel slice, so there
are no cross-core collectives at all and BN statistics are exact.

Per-core layout: SBUF partitions p = b*32 + c_local (128), free = (t,h,w).

Self-contained: hardcodes all shapes; imports concourse from /opt/trn_rl_repo.
"""
import os
import sys

sys.path.insert(0, "/opt/trn_rl_repo")

import numpy as np

import concourse.bass as bass
import concourse.bacc as bacc
import concourse.tile as tile
from concourse import mybir
from concourse.bass_utils import run_bass_kernel_spmd

F32 = mybir.dt.float32
BF16 = mybir.dt.bfloat16
AX = mybir.AxisListType
OP = mybir.AluOpType
AF = mybir.ActivationFunctionType

T, B, C, H, W = 4, 4, 256, 32, 32
CL = 32               # channels per core
NCORES = 8
P = 128               # partitions = B * CL
FT = H * W            # 1024 free per t
F = T * FT            # 4096
INV_SQRT2 = float(np.float32(1.0 / np.sqrt(2.0)))
SQRT2B = float(np.float32(2.0) * np.float32(INV_SQRT2))   # scale for B' fold
TAUS = [0.01, 0.02, 0.02, 0.05]


# --------------------------------------------------------------------------
# device program
# --------------------------------------------------------------------------

def build_module():
    nc = bacc.Bacc("TRN2", target_bir_lowering=False, debug=False)

    def din(name, shape, dt):
        return nc.dram_tensor(name, shape, dt, kind="ExternalInput").ap()

    xin_d = din("xin", [P, F], F32)
    w1_d = din("w1blk", [P, 2 * P], BF16)
    w2_d = din("w2blk", [P, 18 * P], BF16)
    wk_d = din("wkblk", [P, 4 * P], F32)
    selc_d = din("selc", [P, CL], F32)
    selb_d = din("selb", [CL, P], F32)
    bnp_d = din("bnp", [CL, 21], F32)
    cb_d = din("cbias", [P, 2], F32)
    out_d = nc.dram_tensor("out", [P, F], F32, kind="ExternalOutput").ap()

    with tile.TileContext(nc) as tc:
        _emit(tc, nc, xin_d, w1_d, w2_d, wk_d, selc_d, selb_d, bnp_d, cb_d, out_d)
    nc.finalize()
    return nc


def _emit(tc, nc, xin_d, w1_d, w2_d, wk_d, selc_d, selb_d, bnp_d, cb_d, out_d):
    import contextlib

    ctx = contextlib.ExitStack()
    consts = ctx.enter_context(tc.tile_pool(name="consts", bufs=1))
    big = ctx.enter_context(tc.tile_pool(name="big", bufs=1))
    scratch = ctx.enter_context(tc.tile_pool(name="scratch", bufs=2))
    small = ctx.enter_context(tc.tile_pool(name="small", bufs=1))
    psA = ctx.enter_context(tc.tile_pool(name="psA", bufs=2, space="PSUM"))
    psC = ctx.enter_context(tc.tile_pool(name="psC", bufs=4, space="PSUM"))

    # ---- constant loads (gpsimd queue; xin uses the fast queues) ----
    w1_sb = consts.tile([P, 2, P], BF16, tag="w1")
    nc.gpsimd.dma_start(out=w1_sb, in_=w1_d[:].rearrange("p (k n) -> p k n", k=2))
    w2_sb = consts.tile([P, 18, P], BF16, tag="w2")
    nc.gpsimd.dma_start(out=w2_sb, in_=w2_d[:].rearrange("p (k n) -> p k n", k=18))
    wk_sb = consts.tile([P, 4, P], F32, tag="wk")
    nc.gpsimd.dma_start(out=wk_sb, in_=wk_d[:].rearrange("p (k n) -> p k n", k=4))
    selc_sb = consts.tile([P, CL], F32, tag="selc")
    nc.gpsimd.dma_start(out=selc_sb, in_=selc_d[:])
    selb_sb = consts.tile([CL, P], F32, tag="selb")
    nc.gpsimd.dma_start(out=selb_sb, in_=selb_d[:])
    bnp_sb = consts.tile([CL, 21], F32, tag="bnp")
    nc.gpsimd.dma_start(out=bnp_sb, in_=bnp_d[:])
    cb_sb = consts.tile([P, 2], F32, tag="cb")
    nc.gpsimd.dma_start(out=cb_sb, in_=cb_d[:])

    # ---- big tiles ----
    xin = big.tile([P, T, FT], F32, tag="xin")
    v = big.tile([P, FT], F32, tag="v")
    d = big.tile([P, FT], F32, tag="d")
    s = big.tile([P, T, H, W], BF16, tag="s")
    spad = big.tile([P, T, H + 2, W + 2], BF16, tag="spad")
    ulo = big.tile([P, T, H, 16], F32, tag="ulo")
    uhi = big.tile([P, T, H, 16], F32, tag="uhi")
    plo = big.tile([P, T, 16, 16], F32, tag="plo")
    qlo = big.tile([P, T, 16, 16], F32, tag="qlo")
    phi = big.tile([P, T, 16, 16], F32, tag="phi")
    qhi = big.tile([P, T, 16, 16], F32, tag="qhi")
    F32R = mybir.dt.float32r
    cf = big.tile([P, 4, T, 256], F32R, tag="cf")
    hout = big.tile([P, 4, T, 256], F32, tag="hout")
    c1 = big.tile([P, T, H, W], F32, tag="c1")
    c2 = big.tile([P, T, H, W], F32, tag="c2")

    # ---- small stats tiles ----
    pt1 = small.tile([P, 6], F32, tag="pt1")
    ett = small.tile([P, 4, 4], F32, tag="ett")       # E per (band, t)
    mek = small.tile([P, 4, 4], F32, tag="mek")       # maskE per (band, t)
    s1acc = small.tile([P, 4, 4], F32, tag="s1acc")   # sum(cf) per (band, t)
    pt2 = small.tile([P, 8], F32, tag="pt2")
    sr = small.tile([P, 4], F32, tag="sr")
    sq = small.tile([P, 4], F32, tag="sq")
    pt3 = small.tile([P, 2], F32, tag="pt3")
    sc1 = small.tile([P, 8], F32, tag="sc1")
    sc2 = small.tile([P, 8], F32, tag="sc2")
    sq1 = small.tile([P, 4], F32, tag="sq1")
    sq2 = small.tile([P, 4], F32, tag="sq2")
    pt4 = small.tile([P, 4], F32, tag="pt4")
    ab1 = small.tile([P, 4], F32, tag="ab1")          # A'lo A'hi B'lo B'hi
    ab2 = small.tile([P, 8], F32, tag="ab2")          # A2[4] B2[4]
    ab3 = small.tile([P, 4], F32, tag="ab3")          # A_r Btot A1 A2c
    wks = small.tile([P, 4, P], F32R, tag="wks")      # scaled block-diag weights (f32r)
    bdb = small.tile([P, 4], F32, tag="bdb")          # block-diag bias per band
    tmp2 = small.tile([P, 4], F32, tag="tmp2")

    # ========= phase A: load x, LIF =========
    dmaq = [nc.sync, nc.scalar, nc.gpsimd, nc.sync]
    for t in range(T):
        nc.sync.dma_start(out=xin[:, t, 0:512],
                          in_=xin_d[:, t * FT:t * FT + 512])
        nc.scalar.dma_start(out=xin[:, t, 512:FT],
                            in_=xin_d[:, t * FT + 512:(t + 1) * FT])
    sv = s[:].rearrange("p t h w -> p t (h w)")
    SPL = 768
    halves = [(nc.vector, slice(0, SPL)), (nc.gpsimd, slice(SPL, FT))]
    for t in range(T):
        for eng, hs_ in halves:
            xt = xin[:, t, hs_]
            vh, dh = v[:, hs_], d[:, hs_]
            dve = eng is nc.vector
            if t == 0:
                eng.tensor_scalar_mul(vh, xt, 0.5)
            else:
                eng.tensor_sub(dh, xt, vh)
                if dve:
                    eng.scalar_tensor_tensor(
                        out=vh, in0=dh, scalar=0.5, in1=vh, op0=OP.mult, op1=OP.add)
                else:
                    # Pool has no scalar_tensor_tensor: same roundings via
                    # d*=0.5 (exact) then v+=d
                    eng.tensor_scalar_mul(dh, dh, 0.5)
                    eng.tensor_add(vh, vh, dh)
            eng.tensor_single_scalar(
                out=sv[:, t, hs_], in_=vh, scalar=1.0, op=OP.is_ge)
            if t < T - 1:
                if dve:
                    eng.scalar_tensor_tensor(
                        out=vh, in0=vh, scalar=1.0, in1=vh, op0=OP.is_lt, op1=OP.mult)
                else:
                    eng.tensor_single_scalar(out=dh, in_=vh, scalar=1.0, op=OP.is_lt)
                    eng.tensor_mul(vh, vh, dh)
        # padded copy for conv2 taps (per-t so convs can start early);
        # only the border needs zeroing
        if t == 0:
            nc.gpsimd.memset(spad[:, :, 0, :], 0.0)
            nc.gpsimd.memset(spad[:, :, H + 1, :], 0.0)
            nc.gpsimd.memset(spad[:, :, :, 0], 0.0)
            nc.gpsimd.memset(spad[:, :, :, W + 1], 0.0)
        nc.scalar.copy(out=spad[:, t, 1:H + 1, 1:W + 1], in_=s[:, t])

    KSTAGE = int(os.environ.get("KSTAGE", "9"))
    if KSTAGE == 1:
        nc.sync.dma_start(out=out_d[:, 0:FT], in_=v[:])
        ctx.close()
        return

    # ========= phase G1: conv matmuls (PE; overlaps the wavelet path) =========
    # free layout (t, h, w); chunks of 512 = half a t-slice (16 h-rows)
    c1v = c1[:].rearrange("p t h w -> p (t h w)")
    c2v = c2[:].rearrange("p t h w -> p (t h w)")
    KCONV1 = os.environ.get("KCONV1", "1") == "1"
    KCONV2 = os.environ.get("KCONV2", "1") == "1"
    KCOPY = os.environ.get("KCOPY", "actnoacc")
    for half in range(2):
        ps_list = []
        for ck in range(4):
            k = half * 4 + ck
            t, hs = k // 2, (k % 2) * 16
            p1 = psC.tile([P, 512], F32, tag="psc")
            p2 = psC.tile([P, 512], F32, tag="psc")
            if KCONV1:
                for j in range(2):
                    nc.tensor.matmul(p1, w1_sb[:, j], s[:, t, hs:hs + 16, :],
                                     start=(j == 0), stop=(j == 1))
            else:
                nc.vector.memset(p1[:], 0.0)
            if KCONV2:
                for i, (dy, dx) in enumerate([(a, b) for a in range(3) for b in range(3)]):
                    for j in range(2):
                        nc.tensor.matmul(
                            p2, w2_sb[:, 2 * i + j],
                            spad[:, t, hs + dy:hs + dy + 16, dx:dx + 32],
                            start=(i == 0 and j == 0), stop=(i == 8 and j == 1))
            else:
                nc.vector.memset(p2[:], 0.0)
            ps_list.append((k, p1, p2))
        for (k, p1, p2) in ps_list:
            nc.scalar.activation(out=c1v[:, k * 512:(k + 1) * 512], in_=p1,
                                 func=AF.Identity, bias=cb_sb[:, 0:1], scale=1.0)
            nc.scalar.activation(out=c2v[:, k * 512:(k + 1) * 512], in_=p2,
                                 func=AF.Identity, bias=cb_sb[:, 1:2], scale=1.0)
        for q in (0 + 2 * half, 1 + 2 * half):
            sqs = scratch.tile([P, FT], F32, tag="ttrscr")
            nc.scalar.activation(out=sqs[:], in_=c1v[:, q * FT:(q + 1) * FT],
                                 func=AF.Square)
            nc.vector.tensor_scalar(out=d[:], in0=sqs[:], scalar1=0.0, scalar2=0.0,
                                    op0=OP.add, op1=OP.add, accum_out=sq1[:, q:q + 1])
            nc.vector.tensor_scalar(out=d[:], in0=c1v[:, q * FT:(q + 1) * FT],
                                    scalar1=0.0, scalar2=0.0, op0=OP.add, op1=OP.add,
                                    accum_out=sc1[:, q:q + 1])
            sqs2 = scratch.tile([P, FT], F32, tag="ttrscr")
            nc.scalar.activation(out=sqs2[:], in_=c2v[:, q * FT:(q + 1) * FT],
                                 func=AF.Square)
            nc.vector.tensor_scalar(out=d[:], in0=sqs2[:], scalar1=0.0, scalar2=0.0,
                                    op0=OP.add, op1=OP.add, accum_out=sq2[:, q:q + 1])
            nc.vector.tensor_scalar(out=d[:], in0=c2v[:, q * FT:(q + 1) * FT],
                                    scalar1=0.0, scalar2=0.0, op0=OP.add, op1=OP.add,
                                    accum_out=sc2[:, q:q + 1])

    if KSTAGE == 2:
        nc.sync.dma_start(out=out_d[:], in_=c1v[:])
        ctx.close()
        return

    # ========= phase B: Haar along W (unscaled) =========
    KGPS = os.environ.get("KGPS", "1") == "1"
    se = s[:, :, :, 0::2]
    so = s[:, :, :, 1::2]
    nc.vector.tensor_add(ulo[:], se, so)
    (nc.gpsimd if KGPS else nc.vector).tensor_sub(uhi[:], se, so)

    if KSTAGE == 3 and os.environ.get("KSUB") == "a0":
        nc.sync.dma_start(out=out_d[:, 0:2 * FT], in_=ulo[:].rearrange("p t h w -> p (t h w)"))
        ctx.close()
        return
    # ========= phase C: Haar along H (unscaled) + fwd stats =========
    ue, uo = ulo[:, :, 0::2, :], ulo[:, :, 1::2, :]
    he, ho = uhi[:, :, 0::2, :], uhi[:, :, 1::2, :]
    nc.vector.tensor_add(plo[:], ue, uo)
    (nc.gpsimd if KGPS else nc.vector).tensor_sub(qlo[:], ue, uo)
    nc.vector.tensor_add(phi[:], he, ho)
    (nc.gpsimd if KGPS else nc.vector).tensor_sub(qhi[:], he, ho)
    KSUB = os.environ.get("KSUB", "z")
    if KSTAGE == 3 and KSUB == "a":
        nc.sync.dma_start(out=out_d[:, 0:FT], in_=plo[:].rearrange("p t u w -> p (t u w)"))
        ctx.close()
        return
    pv = plo[:].rearrange("p t u w -> p (t u w)")
    nc.vector.tensor_scalar(out=d[:], in0=pv, scalar1=0.0, scalar2=0.0,
                            op0=OP.add, op1=OP.add, accum_out=pt1[:, 0:1])
    pv2 = phi[:].rearrange("p t u w -> p (t u w)")
    nc.vector.tensor_scalar(out=d[:], in0=pv2, scalar1=0.0, scalar2=0.0,
                            op0=OP.add, op1=OP.add, accum_out=pt1[:, 1:2])
    # second moments: sum p^2, q^2  (sum u^2 = (sum p^2 + sum q^2)/2)
    for srcq, col in ((plo, 2), (phi, 3), (qlo, 4), (qhi, 5)):
        sqs = scratch.tile([P, T, 16, 16], F32, tag="ttrscr")
        nc.scalar.activation(out=sqs[:], in_=srcq[:], func=AF.Square)
        nc.vector.tensor_scalar(
            out=d[:], in0=sqs[:].rearrange("p t u w -> p (t u w)"),
            scalar1=0.0, scalar2=0.0, op0=OP.add, op1=OP.add,
            accum_out=pt1[:, col:col + 1])

    if KSTAGE == 3 and KSUB == "b":
        nc.sync.dma_start(out=out_d[:, 0:6], in_=pt1[:])
        ctx.close()
        return
    # fwd stats -> ab1
    st1 = psA.tile([CL, 6], F32, tag="psa")
    nc.tensor.matmul(st1, selc_sb[:], pt1[:], start=True, stop=True)
    sb1 = small.tile([CL, 6], F32, tag="sb1")
    nc.vector.tensor_copy(sb1[:], st1)
    # S2 = sum p^2 + sum q^2  (cols 2:4 + 4:6), then A/B on (CL,2) blocks
    if KSTAGE == 3 and KSUB == "c":
        nc.sync.dma_start(out=out_d[:32, 0:6], in_=sb1[:])
        ctx.close()
        return
    w32 = small.tile([CL, 10], F32, tag="w32")
    _bn_small(nc, small, sb1[:, 0:2], None, sb1[:, 2:4], sb1[:, 4:6],
              n=8192.0, half_s2=True, eps=2e-5,
              g=bnp_sb[:, 0:2], b=bnp_sb[:, 2:4],
              outA=w32[:, 0:2], outB=w32[:, 2:4], w=w32[:, 4:10])
    if KSTAGE == 3 and KSUB == "d":
        nc.sync.dma_start(out=out_d[:32, 0:4], in_=w32[:, 0:4])
        ctx.close()
        return
    bc1 = small.tile([CL, 4], F32, tag="bc1")
    nc.vector.tensor_scalar_mul(bc1[:, 0:2], w32[:, 0:2], INV_SQRT2)   # A'
    nc.vector.tensor_scalar_mul(bc1[:, 2:4], w32[:, 2:4], SQRT2B)      # B'
    bp1 = psA.tile([P, 4], F32, tag="psa")
    nc.tensor.matmul(bp1, selb_sb[:], bc1[:], start=True, stop=True)
    nc.vector.tensor_copy(ab1[:], bp1)

    if KSTAGE == 3:
        nc.sync.dma_start(out=out_d[:, 0:FT], in_=plo[:].rearrange("p t u w -> p (t u w)"))
        nc.sync.dma_start(out=out_d[:, FT:FT + 4], in_=ab1[:])
        ctx.close()
        return

    # ========= phase D: bands (z, gates, energy) =========
    # band order: LL(plo,+B), HL(qlo), LH(phi,+B), HH(qhi)
    band_src = [(plo, 0, True), (qlo, 0, False), (phi, 1, True), (qhi, 1, False)]
    for bi, (pq, ci, has_b) in enumerate(band_src):
        z = scratch.tile([P, T, 256], F32, tag="z")
        zz = scratch.tile([P, T, 256], F32, tag="zz")
        cb_ = scratch.tile([P, T, 256], F32, tag="cband")
        pqv = pq[:].rearrange("p t u w -> p t (u w)")
        a_ap = ab1[:, ci:ci + 1]
        b_ap = ab1[:, 2 + ci:3 + ci]
        if has_b:
            nc.scalar.activation(out=z[:], in_=pqv, func=AF.Identity,
                                 bias=b_ap, scale=a_ap)
            nc.scalar.activation(out=zz[:], in_=pqv, func=AF.Square,
                                 bias=b_ap, scale=a_ap)
        else:
            nc.scalar.activation(out=z[:], in_=pqv, func=AF.Copy, scale=a_ap)
            nc.scalar.activation(out=zz[:], in_=pqv, func=AF.Square, scale=a_ap)
        nc.vector.scalar_tensor_tensor(
            out=cb_[:], in0=zz[:], scalar=0.25, in1=z[:], op0=OP.is_ge, op1=OP.mult)
        cs = scratch.tile([P, T, 256], F32, tag="ttrscr")
        nc.scalar.activation(out=cs[:], in_=cb_[:], func=AF.Square)
        for t in range(T):
            nc.vector.tensor_scalar(
                out=d[:, 0:256], in0=cs[:, t], scalar1=0.0, scalar2=0.0,
                op0=OP.add, op1=OP.add, accum_out=ett[:, bi, t:t + 1])
        thr = float(np.float32(256.0) * np.float32(TAUS[bi]))
        nc.vector.tensor_single_scalar(
            out=mek[:, bi], in_=ett[:, bi], scalar=thr, op=OP.is_gt)
        for t in range(T):
            nc.vector.tensor_scalar(out=cf[:, bi, t], in0=cb_[:, t],
                                    scalar1=mek[:, bi, t:t + 1], scalar2=0.0,
                                    op0=OP.mult, op1=OP.add,
                                    accum_out=s1acc[:, bi, t:t + 1])

    # BN_mul stats -> ab2
    for bi in range(4):
        nc.vector.tensor_reduce(out=pt2[:, bi:bi + 1], in_=s1acc[:, bi],
                                axis=AX.X, op=OP.add)
    nc.vector.tensor_mul(mek[:], mek[:], ett[:])     # maskE * E  (in place)
    for bi in range(4):
        nc.vector.tensor_reduce(out=pt2[:, 4 + bi:5 + bi], in_=mek[:, bi],
                                axis=AX.X, op=OP.add)
    st2 = psA.tile([CL, 8], F32, tag="psa")
    nc.tensor.matmul(st2, selc_sb[:], pt2[:], start=True, stop=True)
    sb2 = small.tile([CL, 8], F32, tag="sb2")
    nc.vector.tensor_copy(sb2[:], st2)
    w32b = small.tile([CL, 20], F32, tag="w32b")
    _bn_small(nc, small, sb2[:, 0:4], sb2[:, 4:8], None, None,
              n=4096.0, half_s2=False, eps=1e-5,
              g=bnp_sb[:, 4:8], b=bnp_sb[:, 8:12],
              outA=w32b[:, 0:4], outB=w32b[:, 4:8], w=w32b[:, 8:20])
    bp2 = psA.tile([P, 8], F32, tag="psa")
    nc.tensor.matmul(bp2, selb_sb[:], w32b[:, 0:8], start=True, stop=True)
    nc.vector.tensor_copy(ab2[:], bp2)

    if KSTAGE == 4:
        nc.sync.dma_start(out=out_d[:], in_=cf[:].rearrange("p k t x -> p (k t x)"))
        ctx.close()
        return

    # ========= phase E: block-diagonal multiply (BN_mul folded in) =========
    cfv = cf[:].rearrange("p k t x -> p k (t x)")
    houtv = hout[:].rearrange("p k t x -> p k (t x)")
    for bi in range(4):
        nc.vector.tensor_scalar_mul(wks[:, bi], wk_sb[:, bi], ab2[:, bi:bi + 1])
        bb = psA.tile([P, 1], F32, tag="psa")
        nc.tensor.matmul(bb, wk_sb[:, bi], ab2[:, 4 + bi:5 + bi], start=True, stop=True)
        nc.vector.tensor_copy(bdb[:, bi:bi + 1], bb)
        for ck in range(2):
            pb = psC.tile([P, 512], F32, tag="psc")
            nc.tensor.matmul(pb, wks[:, bi], cfv[:, bi, ck * 512:(ck + 1) * 512],
                             start=True, stop=True)
            nc.scalar.activation(out=houtv[:, bi, ck * 512:(ck + 1) * 512], in_=pb,
                                 func=AF.Identity, bias=bdb[:, bi:bi + 1], scale=1.0)

    # ========= phase F: inverse Haar (unscaled; x2 absorbed in eps) =========
    # habcd reuses the cf slab (cf fully consumed by the matmuls above);
    # rec reuses the xin slab (dead after LIF)
    habcd = big.tile([P, 4, T, 256], F32, tag="cf")
    rec = big.tile([P, T, H, W], F32, tag="xin")
    LLo, HLo, LHo, HHo = (hout[:, k].rearrange("p t (u w) -> p t u w", u=16)
                          for k in range(4))
    hv = habcd[:].rearrange("p k t (u w) -> p k t u w", u=16)
    hap, ham, hbp, hbm = hv[:, 0], hv[:, 1], hv[:, 2], hv[:, 3]
    nc.vector.tensor_add(hap, LLo, HLo)
    nc.gpsimd.tensor_sub(ham, LLo, HLo)
    nc.vector.tensor_add(hbp, LHo, HHo)
    nc.gpsimd.tensor_sub(hbm, LHo, HHo)
    nc.vector.tensor_add(rec[:, :, 0::2, 0::2], hap, hbp)
    nc.vector.tensor_sub(rec[:, :, 0::2, 1::2], hap, hbp)
    nc.gpsimd.tensor_add(rec[:, :, 1::2, 0::2], ham, hbm)
    nc.gpsimd.tensor_sub(rec[:, :, 1::2, 1::2], ham, hbm)
    hv4 = habcd[:].rearrange("p k t x -> p k (t x)")
    for col, kk in ((0, 0), (1, 1)):      # hap, ham
        nc.vector.tensor_scalar(out=d[:], in0=hv4[:, kk], scalar1=0.0,
                                scalar2=0.0, op0=OP.add, op1=OP.add,
                                accum_out=sr[:, col:col + 1])
    nc.vector.memset(sr[:, 2:4], 0.0)
    for kk in range(4):
        sqs = scratch.tile([P, FT], F32, tag="ttrscr")
        nc.scalar.activation(out=sqs[:], in_=hv4[:, kk], func=AF.Square)
        nc.vector.tensor_scalar(out=d[:], in0=sqs[:], scalar1=0.0, scalar2=0.0,
                                op0=OP.add, op1=OP.add,
                                accum_out=sq[:, kk:kk + 1])
    # sum(rec) = 2*(sum hap + sum ham); sum(rec^2) = 2*sum of squares
    nc.vector.tensor_reduce(out=pt3[:, 0:1], in_=sr[:], axis=AX.X, op=OP.add)
    nc.vector.tensor_scalar_mul(pt3[:, 0:1], pt3[:, 0:1], 2.0)
    nc.vector.tensor_reduce(out=pt3[:, 1:2], in_=sq[:], axis=AX.X, op=OP.add)
    nc.vector.tensor_scalar_mul(pt3[:, 1:2], pt3[:, 1:2], 2.0)

    if KSTAGE == 5:
        nc.sync.dma_start(out=out_d[:], in_=rec[:].rearrange("p t h w -> p (t h w)"))
        ctx.close()
        return

    # ========= phase G2: conv stats (sums computed inline with the convs) =========
    nc.vector.tensor_reduce(out=pt4[:, 0:1], in_=sc1[:, 0:4], axis=AX.X, op=OP.add)
    nc.vector.tensor_reduce(out=pt4[:, 2:3], in_=sc2[:, 0:4], axis=AX.X, op=OP.add)
    nc.vector.tensor_reduce(out=pt4[:, 1:2], in_=sq1[:, 0:4], axis=AX.X, op=OP.add)
    nc.vector.tensor_reduce(out=pt4[:, 3:4], in_=sq2[:, 0:4], axis=AX.X, op=OP.add)

    # inv + conv stats in one round -> A_r/B_r, A1/B1, A2c/B2c
    pt34 = small.tile([P, 6], F32, tag="pt34")
    nc.vector.tensor_copy(pt34[:, 0:1], pt3[:, 0:1])
    nc.vector.tensor_copy(pt34[:, 1:2], pt4[:, 0:1])
    nc.vector.tensor_copy(pt34[:, 2:3], pt4[:, 2:3])
    nc.vector.tensor_copy(pt34[:, 3:4], pt3[:, 1:2])
    nc.vector.tensor_copy(pt34[:, 4:5], pt4[:, 1:2])
    nc.vector.tensor_copy(pt34[:, 5:6], pt4[:, 3:4])
    st3 = psA.tile([CL, 6], F32, tag="psa")
    nc.tensor.matmul(st3, selc_sb[:], pt34[:], start=True, stop=True)
    sb3 = small.tile([CL, 6], F32, tag="sb3")
    nc.vector.tensor_copy(sb3[:], st3)
    w32c = small.tile([CL, 15], F32, tag="w32c")
    _bn_small(nc, small, sb3[:, 0:3], sb3[:, 3:6], None, None,
              n=16384.0, half_s2=False, eps=bnp_sb[:, 18:21],
              g=bnp_sb[:, 12:15], b=bnp_sb[:, 15:18],
              outA=w32c[:, 0:3], outB=w32c[:, 3:6], w=w32c[:, 6:15])

    # pack [A_r, Btot, A1, A2c] and broadcast
    bc3 = small.tile([CL, 4], F32, tag="bc3")
    nc.vector.tensor_copy(bc3[:, 0:1], w32c[:, 0:1])
    nc.vector.tensor_reduce(out=bc3[:, 1:2], in_=w32c[:, 3:6], axis=AX.X, op=OP.add)
    nc.vector.tensor_copy(bc3[:, 2:4], w32c[:, 1:3])
    bp3 = psA.tile([P, 4], F32, tag="psa")
    nc.tensor.matmul(bp3, selb_sb[:], bc3[:], start=True, stop=True)
    nc.vector.tensor_copy(ab3[:], bp3)

    # ========= phase H: final combine + store =========
    recv = rec[:].rearrange("p t h w -> p (t h w)")
    FSPL = 768
    for t in range(T):
        cols = slice(t * FT, (t + 1) * FT)
        nc.scalar.activation(out=recv[:, cols], in_=recv[:, cols], func=AF.Identity,
                             bias=ab3[:, 1:2], scale=ab3[:, 0:1])
        a_, b_ = t * FT, t * FT + FSPL
        nc.vector.scalar_tensor_tensor(
            out=recv[:, a_:b_], in0=c1v[:, a_:b_], scalar=ab3[:, 2:3],
            in1=recv[:, a_:b_], op0=OP.mult, op1=OP.add)
        nc.vector.scalar_tensor_tensor(
            out=recv[:, a_:b_], in0=c2v[:, a_:b_], scalar=ab3[:, 3:4],
            in1=recv[:, a_:b_], op0=OP.mult, op1=OP.add)
        a_, b_ = t * FT + FSPL, (t + 1) * FT
        pscr = scratch.tile([P, FT - FSPL], F32, tag="poolscr")
        nc.gpsimd.tensor_scalar_mul(pscr[:], c1v[:, a_:b_], ab3[:, 2:3])
        nc.gpsimd.tensor_add(recv[:, a_:b_], recv[:, a_:b_], pscr[:])
        pscr2 = scratch.tile([P, FT - FSPL], F32, tag="poolscr")
        nc.gpsimd.tensor_scalar_mul(pscr2[:], c2v[:, a_:b_], ab3[:, 3:4])
        nc.gpsimd.tensor_add(recv[:, a_:b_], recv[:, a_:b_], pscr2[:])
        dmaq[t].dma_start(out=out_d[:, t * FT:(t + 1) * FT], in_=recv[:, cols])

    ctx.close()


def _bn_small(nc, pool, S1, S2, S2a, S2b, n, half_s2, eps, g, b, outA, outB, w):
    """BN affine params on CL partitions, vectorized over k adjacent columns.

    S1: (CL,k) raw sums; S2 (or S2a+S2b when half_s2): raw sums of squares.
    outA = g * rsqrt(var + eps); outB = b - outA * mu.
    w: (CL, 3k) workspace.
    """
    k = S1.shape[1]
    nmu, ex2, t0 = w[:, 0:k], w[:, k:2 * k], w[:, 2 * k:3 * k]
    nc.vector.tensor_scalar_mul(nmu, S1, -1.0 / n)
    if half_s2:
        nc.vector.tensor_add(ex2, S2a, S2b)
        nc.vector.tensor_scalar_mul(ex2, ex2, 0.5 / n)
    else:
        nc.vector.tensor_scalar_mul(ex2, S2, 1.0 / n)
    nc.vector.tensor_mul(t0, nmu, nmu)
    nc.vector.tensor_sub(ex2, ex2, t0)                      # var
    if isinstance(eps, float):
        nc.vector.tensor_scalar_add(ex2, ex2, eps)
    else:
        nc.vector.tensor_add(ex2, ex2, eps)                 # per-column eps AP
    nc.scalar.sqrt(t0, ex2)
    nc.vector.reciprocal(t0, t0)                            # rsqrt(var+eps)
    nc.vector.tensor_mul(outA, g, t0)
    # B = b - A*mu ; nmu = -mu so B = (A * nmu) + b, done per column since
    # the STT scalar must be a single per-partition value
    for j in range(k):
        nc.vector.scalar_tensor_tensor(
            out=outB[:, j:j + 1], in0=outA[:, j:j + 1], scalar=nmu[:, j:j + 1],
            in1=b[:, j:j + 1], op0=OP.mult, op1=OP.add)


# --------------------------------------------------------------------------
# host wrapper
# --------------------------------------------------------------------------

_NC = None


def _get_module():
    global _NC
    if _NC is None:
        _NC = build_module()
    return _NC


def _host_prep(inputs):
    """Build the 8 per-core input maps from full inputs."""
    x = np.asarray(inputs["x"], np.float32)
    haar_weight = np.asarray(inputs["haar_weight"], np.float32)
    conv1_w = np.asarray(inputs["conv1_w"], np.float32)
    conv1_b = np.asarray(inputs["conv1_b"], np.float32)
    conv2_w = np.asarray(inputs["conv2_w"], np.float32)
    conv2_b = np.asarray(inputs["conv2_b"], np.float32)

    # block-diag selector matrices (shared)
    selc = np.zeros((P, CL), np.float32)
    selc[np.arange(P), np.arange(P) % CL] = 1.0
    selb = np.ascontiguousarray(selc.T)

    # conv stationaries: lhsT[(b,i), (b,o)] = W[o,i] within each 16-group
    def blockdiag16(w_oi):  # (16,16) -> (128,128) lhsT
        m = np.zeros((P, P), np.float32)
        for g in range(8):      # 4 b * 2 groups
            m[g * 16:(g + 1) * 16, g * 16:(g + 1) * 16] = w_oi.T
        return m

    def hilo(m):  # fp32 (..., P) -> bf16 hi and lo parts stacked on a new axis
        hi = _to_bf16(m)
        lo = _to_bf16(m - hi.astype(np.float32))
        return hi, lo

    w1blk = blockdiag16(conv1_w[:, :, 0, 0]).astype(np.float32)
    w1hi, w1lo = hilo(w1blk)
    w1blk_bf = np.stack([w1hi, w1lo])                      # (2, P, P)
    w2blk = np.stack([blockdiag16(conv2_w[:, :, dy, dx])
                      for dy in range(3) for dx in range(3)])
    w2hi, w2lo = hilo(w2blk)
    w2blk_bf = np.empty((18, P, P), dtype=w2hi.dtype)
    w2blk_bf[0::2] = w2hi
    w2blk_bf[1::2] = w2lo

    cbias = np.zeros((P, 2), np.float32)
    cbias[:, 0] = np.tile(conv1_b, 8)
    cbias[:, 1] = np.tile(conv2_b, 8)

    in_maps = []
    for dd in range(NCORES):
        c0 = CL * dd
        sl = slice(c0, c0 + CL)
        x_core = np.ascontiguousarray(
            x[:, :, sl].transpose(1, 2, 0, 3, 4)).reshape(P, F)
        # block-diag stationaries: lhsT[(b,g,d),(b,g,m)] = Wk[d,m]
        wkblk = np.zeros((4, P, P), np.float32)
        for k in range(4):
            wk = haar_weight[4 * k + dd // 2]
            for g in range(8):
                wkblk[k, g * 16:(g + 1) * 16, g * 16:(g + 1) * 16] = wk
        # (P, 4, P) layout to match tile [P, 4, P]
        wk_host = np.ascontiguousarray(wkblk.transpose(1, 0, 2)).reshape(P, 4 * P)
        w2_host = np.ascontiguousarray(w2blk_bf.transpose(1, 0, 2)).reshape(P, 18 * P)

        bnp = np.zeros((CL, 21), np.float32)
        bnp[:, 0] = inputs["bn_fwd_g"][sl]
        bnp[:, 1] = inputs["bn_fwd_g"][C + c0:C + c0 + CL]
        bnp[:, 2] = inputs["bn_fwd_b"][sl]
        bnp[:, 3] = inputs["bn_fwd_b"][C + c0:C + c0 + CL]
        gm = np.asarray(inputs["bn_mul_g"], np.float32).reshape(4, C)[:, sl]
        bm = np.asarray(inputs["bn_mul_b"], np.float32).reshape(4, C)[:, sl]
        bnp[:, 4:8] = gm.T
        bnp[:, 8:12] = bm.T
        bnp[:, 12] = inputs["bn_inv_g"][sl]
        bnp[:, 13] = inputs["bn_c1_g"][sl]
        bnp[:, 14] = inputs["bn_c2_g"][sl]
        bnp[:, 15] = inputs["bn_inv_b"][sl]
        bnp[:, 16] = inputs["bn_c1_b"][sl]
        bnp[:, 17] = inputs["bn_c2_b"][sl]
        bnp[:, 18] = 4e-5   # bn_inv eps (x4: unscaled inverse haar)
        bnp[:, 19] = 1e-5   # bn_c1 eps
        bnp[:, 20] = 1e-5   # bn_c2 eps

        in_maps.append({
            "xin": x_core,
            "w1blk": np.ascontiguousarray(
                w1blk_bf.transpose(1, 0, 2)).reshape(P, 2 * P),
            "w2blk": w2_host,
            "wkblk": wk_host,
            "selc": selc,
            "selb": selb,
            "bnp": np.ascontiguousarray(bnp),
            "cbias": cbias,
        })
    return in_maps


def _to_bf16(a):
    return np.asarray(a, dtype=mybir.dt.np(BF16))


def _assemble(results):
    out = np.zeros((T, B, C, H, W), np.float32)
    for dd in range(NCORES):
        oc = np.asarray(results[dd]["out"]).reshape(B, CL, T, H, W)
        out[:, :, CL * dd:CL * (dd + 1)] = oc.transpose(2, 0, 1, 3, 4)
    return out


def kernel(**inputs):
    nc = _get_module()
    in_maps = _host_prep(inputs)
    res = run_bass_kernel_spmd(nc, in_maps, list(range(NCORES)))
    return _assemble(res.results)


if __name__ == "__main__":
    # smoke test with random inputs
    rng = np.random.default_rng(0)
    inputs = {
        "x": rng.standard_normal((T, B, C, H, W), np.float32),
        "haar_weight": 0.02 * rng.standard_normal((16, 16, 16), np.float32),
        "conv1_w": 0.1 * rng.standard_normal((16, 16, 1, 1), np.float32),
        "conv1_b": np.zeros(16, np.float32),
        "conv2_w": 0.05 * rng.standard_normal((16, 16, 3, 3), np.float32),
        "conv2_b": np.zeros(16, np.float32),
        "bn_fwd_g": np.ones(512, np.float32), "bn_fwd_b": np.zeros(512, np.float32),
        "bn_mul_g": np.ones(1024, np.float32), "bn_mul_b": np.zeros(1024, np.float32),
        "bn_inv_g": np.ones(256, np.float32), "bn_inv_b": np.zeros(256, np.float32),
        "bn_c1_g": np.ones(256, np.float32), "bn_c1_b": np.zeros(256, np.float32),
        "bn_c2_g": np.ones(256, np.float32), "bn_c2_b": np.zeros(256, np.float32),
    }
    out = kernel(**inputs)
    print("out", out.shape, out.dtype, np.abs(out).mean())



# revision 24
# speedup vs baseline: 1.3858x; 1.3858x over previous
"""Trainium2 Bass kernel for nn_FATMSparse (spiking Haar-wavelet network).

Sharding: the 256 channels are split 32-per-core across 8 cores. Every
stage of the network (LIF, Haar transforms, all five training-mode
BatchNorms, the per-16-channel block-diagonal mixes and both grouped
convolutions) is exactly local to an aligned 32-channel slice, so there
are no cross-core collectives at all and BN statistics are exact.

Per-core layout: SBUF partitions p = b*32 + c_local (128), free = (t,h,w).

v2 notes (vs the 93us baseline):
 - convs run on single-precision bf16 weights (one matmul per tap);
 - spikes are written into a halo-padded bf16 tile so conv taps and the
   Haar transform read it directly (no separate padded copy);
 - LIF uses the doubled-potential recurrence w=2v (3 ops/step);
 - all BN statistics ride accum_out side-outputs (Act Square+accum, DVE
   tensor_tensor_reduce, tensor_scalar accum) instead of square+reduce
   passes; level-1 Haar stats use the spike-integer identities
   sum(ulo^2)=sum(s)+2*cnt{ulo=2}, sum(uhi^2)=sum(s)-2*cnt{ulo=2};
 - conv bias is dropped entirely (training-mode BN cancels it).

Self-contained: hardcodes all shapes; imports concourse from /opt/trn_rl_repo.
"""
import os
import sys

sys.path.insert(0, "/opt/trn_rl_repo")

import numpy as np

import concourse.bass as bass
import concourse.bacc as bacc
import concourse.tile as tile
from concourse import mybir
from concourse.bass_utils import run_bass_kernel_spmd

F32 = mybir.dt.float32
F32R = mybir.dt.float32r
BF16 = mybir.dt.bfloat16
AX = mybir.AxisListType
OP = mybir.AluOpType
AF = mybir.ActivationFunctionType

T, B, C, H, W = 4, 4, 256, 32, 32
CL = 32               # channels per core
NCORES = 8
P = 128               # partitions = B * CL
FT = H * W            # 1024 free per t
F = T * FT            # 4096
INV_SQRT2 = float(np.float32(1.0 / np.sqrt(2.0)))
SQRT2B = float(np.float32(2.0) * np.float32(INV_SQRT2))   # scale for B' fold
TAUS = [0.01, 0.02, 0.02, 0.05]
RD = 19               # LIF/DVE takes h-rows [0, RD), Pool takes [RD, H)


def build_module():
    nc = bacc.Bacc("TRN2", target_bir_lowering=False, debug=False)

    def din(name, shape, dt):
        return nc.dram_tensor(name, shape, dt, kind="ExternalInput").ap()

    xin_d = din("xin", [P, F], F32)
    w1_d = din("w1blk", [P, P], BF16)
    w2_d = din("w2blk", [P, 9 * P], BF16)
    wk_d = din("wkblk", [P, 4 * P], F32)
    selc_d = din("selc", [P, CL], F32)
    selb_d = din("selb", [CL, P], F32)
    bnp_d = din("bnp", [CL, 21], F32)
    out_d = nc.dram_tensor("out", [P, F], F32, kind="ExternalOutput").ap()

    with tile.TileContext(nc) as tc:
        _emit(tc, nc, xin_d, w1_d, w2_d, wk_d, selc_d, selb_d, bnp_d, out_d)
    nc.finalize()
    return nc


def _emit(tc, nc, xin_d, w1_d, w2_d, wk_d, selc_d, selb_d, bnp_d, out_d):
    import contextlib

    ctx = contextlib.ExitStack()
    consts = ctx.enter_context(tc.tile_pool(name="consts", bufs=1))
    big = ctx.enter_context(tc.tile_pool(name="big", bufs=1))
    scratch = ctx.enter_context(tc.tile_pool(name="scratch", bufs=2))
    small = ctx.enter_context(tc.tile_pool(name="small", bufs=1))
    psB = ctx.enter_context(tc.tile_pool(name="psB", bufs=3, space="PSUM"))
    psS = ctx.enter_context(tc.tile_pool(name="psS", bufs=2, space="PSUM"))

    # ---- constant loads (gpsimd queue) ----
    w1_sb = consts.tile([P, P], BF16, tag="w1")
    nc.gpsimd.dma_start(out=w1_sb, in_=w1_d[:])
    w2_sb = consts.tile([P, 9, P], BF16, tag="w2")
    nc.gpsimd.dma_start(out=w2_sb, in_=w2_d[:].rearrange("p (k n) -> p k n", k=9))
    wk_sb = consts.tile([P, 4, P], F32, tag="wk")
    nc.gpsimd.dma_start(out=wk_sb, in_=wk_d[:].rearrange("p (k n) -> p k n", k=4))
    selc_sb = consts.tile([P, CL], F32, tag="selc")
    nc.gpsimd.dma_start(out=selc_sb, in_=selc_d[:])
    selb_sb = consts.tile([CL, P], F32, tag="selb")
    nc.gpsimd.dma_start(out=selb_sb, in_=selb_d[:])
    bnp_sb = consts.tile([CL, 21], F32, tag="bnp")
    nc.gpsimd.dma_start(out=bnp_sb, in_=bnp_d[:])

    # ---- big tiles ----
    xin = big.tile([P, T, FT], F32, tag="xin")
    wv = big.tile([P, H, W], F32, tag="wv")        # 2*v LIF state
    mv = big.tile([P, H, W], F32, tag="mv")        # masked w
    s = big.tile([P, T, H + 2, W + 2], BF16, tag="s")
    ulo = big.tile([P, T, H, 16], BF16, tag="ulo")
    uhi = big.tile([P, T, H, 16], BF16, tag="uhi")
    plo = big.tile([P, T, 16, 16], BF16, tag="plo")
    qlo = big.tile([P, T, 16, 16], BF16, tag="qlo")
    phi = big.tile([P, T, 16, 16], BF16, tag="phi")
    qhi = big.tile([P, T, 16, 16], BF16, tag="qhi")
    zb = big.tile([P, 4, T, 256], F32, tag="zb")   # z per band
    cf = big.tile([P, 4, T, 256], F32R, tag="cf")  # gated coeffs / matmul rhs
    hout = big.tile([P, 4, T, 256], F32, tag="hout")
    c1 = big.tile([P, T, FT], F32, tag="c1")
    c2 = big.tile([P, T, FT], F32, tag="c2")

    # ---- small stats tiles ----
    su_t = small.tile([P, 4], F32, tag="su_t")     # sum(ulo) per t
    cn_t = small.tile([P, 4], F32, tag="cn_t")     # cnt{ulo==2} per t
    sh_t = small.tile([P, 4], F32, tag="sh_t")     # sum(uhi) per t
    pt1 = small.tile([P, 3], F32, tag="pt1")
    sc1 = small.tile([P, 4], F32, tag="sc1")       # conv sums per t
    sq1 = small.tile([P, 4], F32, tag="sq1")
    sc2 = small.tile([P, 4], F32, tag="sc2")
    sq2 = small.tile([P, 4], F32, tag="sq2")
    ett = small.tile([P, 4, 4], F32, tag="ett")    # band energy per (band, t)
    mek = small.tile([P, 4, 4], F32, tag="mek")
    s1acc = small.tile([P, 4, 4], F32, tag="s1acc")
    pt2 = small.tile([P, 8], F32, tag="pt2")
    sr = small.tile([P, 2], F32, tag="sr")         # sum(hap), sum(ham)
    sq = small.tile([P, 4], F32, tag="sq")         # sumsq of hap/ham/hbp/hbm
    pt3 = small.tile([P, 2], F32, tag="pt3")
    pt4 = small.tile([P, 4], F32, tag="pt4")
    ab1 = small.tile([P, 4], F32, tag="ab1")       # A'lo A'hi B'lo B'hi
    ab2 = small.tile([P, 8], F32, tag="ab2")
    ab3 = small.tile([P, 4], F32, tag="ab3")       # A_r Btot A1 A2c
    wks = small.tile([P, 4, P], F32R, tag="wks")
    bdb = small.tile([P, 4], F32, tag="bdb")

    xv = xin[:].rearrange("p t (h w) -> p t h w", w=W)
    KSTAGE = int(os.environ.get("KSTAGE", "9"))

    # halo borders of s zeroed once (rows 0 and H+1, cols 0 and W+1)
    nc.gpsimd.memset(s[:, :, 0, :], 0.0)
    nc.gpsimd.memset(s[:, :, H + 1, :], 0.0)
    nc.gpsimd.memset(s[:, :, 1:H + 1, 0], 0.0)
    nc.gpsimd.memset(s[:, :, 1:H + 1, W + 1], 0.0)

    # input DMAs (two fast queues, one per half-t)
    for t in range(T):
        nc.sync.dma_start(out=xin[:, t, 0:512], in_=xin_d[:, t * FT:t * FT + 512])
        nc.scalar.dma_start(out=xin[:, t, 512:FT],
                            in_=xin_d[:, t * FT + 512:(t + 1) * FT])

    lif_split = [(nc.vector, 0, RD), (nc.gpsimd, RD, H)]

    def conv_t(t, evict2):
        """PE matmuls + eviction/stat side-ops for time step t."""
        p1 = psB.tile([P, 1024], F32, tag="psw")
        p2 = psB.tile([P, 1024], F32, tag="psw")
        for ck in range(2):
            hs = ck * 16
            nc.tensor.matmul(p1[:, ck * 512:(ck + 1) * 512], w1_sb[:],
                             s[:, t, 1 + hs:17 + hs, 1:W + 1],
                             start=True, stop=True)
        for ck in range(2):
            hs = ck * 16
            for i, (dy, dx) in enumerate([(a, b) for a in range(3)
                                          for b in range(3)]):
                nc.tensor.matmul(p2[:, ck * 512:(ck + 1) * 512], w2_sb[:, i],
                                 s[:, t, hs + dy:hs + dy + 16, dx:dx + 32],
                                 start=(i == 0), stop=(i == 8))
        # evictions with fused sums; sumsq straight off PSUM
        nc.scalar.activation(out=c1[:, t], in_=p1[:], func=AF.Copy,
                             accum_out=sc1[:, t:t + 1])
        dmpA = scratch.tile([P, 1024], F32, tag="dmpA")
        nc.scalar.activation(out=dmpA[:], in_=p1[:], func=AF.Square,
                             accum_out=sq1[:, t:t + 1])
        dmpA2 = scratch.tile([P, 1024], F32, tag="dmpA")
        nc.scalar.activation(out=dmpA2[:], in_=p2[:], func=AF.Square,
                             accum_out=sq2[:, t:t + 1])
        if evict2 is nc.scalar:
            nc.scalar.activation(out=c2[:, t], in_=p2[:], func=AF.Copy,
                                 accum_out=sc2[:, t:t + 1])
        else:
            evict2.tensor_scalar(out=c2[:, t], in0=p2[:], scalar1=0.0,
                                 scalar2=0.0, op0=OP.add, op1=OP.add,
                                 accum_out=sc2[:, t:t + 1])

    # ========= phase A+B+C: LIF, Haar W/H, fwd stats (DVE/Pool) =========
    for t in range(T):
        for eng, r0, r1 in lif_split:
            dve = eng is nc.vector
            xt = xv[:, t, r0:r1]
            wt, mt = wv[:, r0:r1], mv[:, r0:r1]
            st = s[:, t, 1 + r0:1 + r1, 1:W + 1]
            if t == 0:
                eng.tensor_single_scalar(out=st, in_=xt, scalar=2.0, op=OP.is_ge)
                if dve:
                    eng.scalar_tensor_tensor(out=mt, in0=xt, scalar=2.0,
                                             in1=xt, op0=OP.is_lt, op1=OP.mult)
                else:
                    # Pool has no scalar_tensor_tensor on real HW
                    eng.tensor_single_scalar(out=mt, in_=xt, scalar=2.0,
                                             op=OP.is_lt)
                    eng.tensor_mul(mt, xt, mt)
            else:
                if dve:
                    eng.scalar_tensor_tensor(out=wt, in0=mt, scalar=0.5,
                                             in1=xt, op0=OP.mult, op1=OP.add)
                else:
                    eng.tensor_scalar_mul(mt, mt, 0.5)
                    eng.tensor_add(wt, mt, xt)
                eng.tensor_single_scalar(out=st, in_=wt, scalar=2.0, op=OP.is_ge)
                if t < T - 1:
                    if dve:
                        eng.scalar_tensor_tensor(out=mt, in0=wt, scalar=2.0,
                                                 in1=wt, op0=OP.is_lt,
                                                 op1=OP.mult)
                    else:
                        eng.tensor_single_scalar(out=mt, in_=wt, scalar=2.0,
                                                 op=OP.is_lt)
                        eng.tensor_mul(mt, wt, mt)
        # Haar along W (unscaled, bf16-exact): ulo=se+so, uhi=se-so
        se = s[:, t, 1:H + 1, 1:W + 1:2]
        so = s[:, t, 1:H + 1, 2:W + 2:2]
        nc.vector.tensor_add(ulo[:, t], se, so)
        nc.gpsimd.tensor_sub(uhi[:, t], se, so)
        # Haar along H: plo/qlo on DVE (bf16 2x), phi/qhi on Pool
        ue, uo = ulo[:, t, 0::2, :], ulo[:, t, 1::2, :]
        he, ho = uhi[:, t, 0::2, :], uhi[:, t, 1::2, :]
        nc.vector.tensor_add(plo[:, t], ue, uo)
        nc.vector.tensor_sub(qlo[:, t], ue, uo)
        nc.gpsimd.tensor_add(phi[:, t], he, ho)
        nc.gpsimd.tensor_sub(qhi[:, t], he, ho)

    # fwd stats (single full-tensor bf16 accums; spike-integer identities):
    # sum(l1_lo raw) = sum(ulo) = sum(s); cnt2 = #{ulo==2};
    # sum(ulo^2) = sum(s)+2*cnt2 ; sum(uhi^2) = sum(s)-2*cnt2
    ulof = ulo[:].rearrange("p t h w -> p (t h w)")
    uhif = uhi[:].rearrange("p t h w -> p (t h w)")
    dv = scratch.tile([P, 2048], BF16, tag="dmpV")
    nc.vector.tensor_scalar(out=dv[:], in0=ulof, scalar1=0.0,
                            scalar2=0.0, op0=OP.add, op1=OP.add,
                            accum_out=pt1[:, 0:1])
    dv2 = scratch.tile([P, 2048], BF16, tag="dmpV")
    nc.vector.tensor_scalar(out=dv2[:], in0=uhif, scalar1=0.0,
                            scalar2=0.0, op0=OP.add, op1=OP.add,
                            accum_out=pt1[:, 1:2])
    dv3 = scratch.tile([P, 2048], BF16, tag="dmpV")
    nc.vector.tensor_scalar(out=dv3[:], in0=ulof, scalar1=2.0,
                            scalar2=0.0, op0=OP.is_ge, op1=OP.add,
                            accum_out=pt1[:, 2:3])

    if KSTAGE == 1:
        nc.sync.dma_start(out=out_d[:, 0:FT],
                          in_=wv[:].rearrange("p h w -> p (h w)"))
        ctx.close()
        return

    # ========= convs t0/t1, then fwd BN params, then convs t2/t3 =========
    # Ordering puts st1/bp1 into the PE queue between conv-t1 and conv-t2 so
    # the band phase can start while conv-t2/t3 still run; late c2 evictions
    # go to Act so DVE/Pool are free for the band phase.
    conv_t(0, nc.vector)
    conv_t(1, nc.vector)
    st1 = psS.tile([CL, 3], F32, tag="pss")
    nc.tensor.matmul(st1, selc_sb[:], pt1[:], start=True, stop=True)
    stc = small.tile([CL, 3], F32, tag="stc")
    nc.vector.tensor_copy(stc[:], st1)
    sb1 = small.tile([CL, 4], F32, tag="sb1")
    # [S1lo, S1hi, S2lo, S2hi]; S2lo = S + 2*cnt2, S2hi = S - 2*cnt2
    nc.vector.tensor_copy(sb1[:, 0:2], stc[:, 0:2])
    nc.vector.scalar_tensor_tensor(out=sb1[:, 2:3], in0=stc[:, 2:3], scalar=2.0,
                                   in1=stc[:, 0:1], op0=OP.mult, op1=OP.add)
    nc.vector.scalar_tensor_tensor(out=sb1[:, 3:4], in0=stc[:, 2:3], scalar=-2.0,
                                   in1=stc[:, 0:1], op0=OP.mult, op1=OP.add)
    w32 = small.tile([CL, 10], F32, tag="w32")
    _bn_small(nc, small, sb1[:, 0:2], sb1[:, 2:4], None, None,
              n=8192.0, half_s2=False, eps=2e-5,
              g=bnp_sb[:, 0:2], b=bnp_sb[:, 2:4],
              outA=w32[:, 0:2], outB=w32[:, 2:4], w=w32[:, 4:10])
    bc1 = small.tile([CL, 4], F32, tag="bc1")
    nc.vector.tensor_scalar_mul(bc1[:, 0:2], w32[:, 0:2], INV_SQRT2)   # A'
    nc.vector.tensor_scalar_mul(bc1[:, 2:4], w32[:, 2:4], SQRT2B)      # B'
    bp1 = psS.tile([P, 4], F32, tag="pss")
    nc.tensor.matmul(bp1, selb_sb[:], bc1[:], start=True, stop=True)
    nc.vector.tensor_copy(ab1[:], bp1)
    conv_t(2, nc.scalar)
    conv_t(3, nc.scalar)

    if KSTAGE == 3:
        nc.sync.dma_start(out=out_d[:, 0:FT],
                          in_=plo[:].rearrange("p t u w -> p (t u w)"))
        nc.sync.dma_start(out=out_d[:, FT:FT + 4], in_=ab1[:])
        ctx.close()
        return

    # ========= bands: z, gate, energy, cf =========
    # band order: LL(plo,+B), HL(qlo), LH(phi,+B), HH(qhi)
    band_src = [(plo, 0, True), (qlo, 0, False), (phi, 1, True), (qhi, 1, False)]
    # z+m on DVE for all bands; u on pool for LH/HH, DVE for LL/HL;
    # E via DVE TTR (LL/HL) and Act Square+accum (LH/HH)
    for bi, (pq, ci, has_b) in enumerate(band_src):
        pqv = pq[:].rearrange("p t u w -> p t (u w)")
        a_ap = ab1[:, ci:ci + 1]
        z = zb[:, bi]
        zf = z.rearrange("p t x -> p (t x)")
        on_dve = bi < 2
        ev, ep = (nc.vector, nc.gpsimd)
        if has_b:
            (ev if on_dve else ep).tensor_scalar(
                out=z, in0=pqv, scalar1=a_ap, scalar2=ab1[:, 2 + ci:3 + ci],
                op0=OP.mult, op1=OP.add)
        else:
            (ev if on_dve else ep).tensor_scalar(
                out=z, in0=pqv, scalar1=a_ap, scalar2=0.0,
                op0=OP.mult, op1=OP.add)
        zz = scratch.tile([P, T, 256], F32, tag="mband" + str(bi % 2))
        (ev if on_dve else ep).tensor_mul(zz[:], z, z)
        u = cf[:, bi]
        if on_dve:
            nc.vector.scalar_tensor_tensor(out=u, in0=zz[:], scalar=0.25,
                                           in1=z, op0=OP.is_ge, op1=OP.mult)
        else:
            ep.tensor_single_scalar(out=zz[:], in_=zz[:], scalar=0.25,
                                    op=OP.is_ge)
            ep.tensor_mul(u, zz[:], z)
        for t in range(T):
            da = scratch.tile([P, 256], F32, tag="dmpA2")
            nc.scalar.activation(out=da[:], in_=u[:, t], func=AF.Square,
                                 accum_out=ett[:, bi, t:t + 1])
        thr = float(np.float32(256.0) * np.float32(TAUS[bi]))
        (ev if on_dve else ep).tensor_single_scalar(
            out=mek[:, bi], in_=ett[:, bi], scalar=thr, op=OP.is_gt)
        for t in range(T):
            if on_dve:
                nc.vector.tensor_scalar(
                    out=u[:, t], in0=u[:, t], scalar1=mek[:, bi, t:t + 1],
                    scalar2=0.0, op0=OP.mult, op1=OP.add,
                    accum_out=s1acc[:, bi, t:t + 1])
            else:
                # Act fuses the mask multiply with the s1 accumulation
                nc.scalar.activation(out=u[:, t], in_=u[:, t], func=AF.Copy,
                                     scale=mek[:, bi, t:t + 1],
                                     accum_out=s1acc[:, bi, t:t + 1])

    # BN_mul stats -> ab2
    for bi in range(4):
        nc.vector.tensor_reduce(out=pt2[:, bi:bi + 1], in_=s1acc[:, bi],
                                axis=AX.X, op=OP.add)
    nc.vector.tensor_mul(mek[:], mek[:], ett[:])     # maskE * E  (in place)
    for bi in range(4):
        nc.vector.tensor_reduce(out=pt2[:, 4 + bi:5 + bi], in_=mek[:, bi],
                                axis=AX.X, op=OP.add)
    st2 = psS.tile([CL, 8], F32, tag="pss")
    nc.tensor.matmul(st2, selc_sb[:], pt2[:], start=True, stop=True)
    sb2 = small.tile([CL, 8], F32, tag="sb2")
    nc.vector.tensor_copy(sb2[:], st2)
    w32b = small.tile([CL, 20], F32, tag="w32b")
    _bn_small(nc, small, sb2[:, 0:4], sb2[:, 4:8], None, None,
              n=4096.0, half_s2=False, eps=1e-5,
              g=bnp_sb[:, 4:8], b=bnp_sb[:, 8:12],
              outA=w32b[:, 0:4], outB=w32b[:, 4:8], w=w32b[:, 8:20])
    bp2 = psS.tile([P, 8], F32, tag="pss")
    nc.tensor.matmul(bp2, selb_sb[:], w32b[:, 0:8], start=True, stop=True)
    nc.vector.tensor_copy(ab2[:], bp2)

    if KSTAGE == 4:
        nc.sync.dma_start(out=out_d[:],
                          in_=cf[:].rearrange("p k t x -> p (k t x)"))
        ctx.close()
        return

    # ========= block-diagonal multiply (BN_mul folded in) =========
    cfv = cf[:].rearrange("p k t x -> p k (t x)")
    for bi in range(4):
        nc.vector.tensor_scalar_mul(wks[:, bi], wk_sb[:, bi], ab2[:, bi:bi + 1])
        bb = psS.tile([P, 1], F32, tag="pss")
        nc.tensor.matmul(bb, wk_sb[:, bi], ab2[:, 4 + bi:5 + bi],
                         start=True, stop=True)
        nc.vector.tensor_copy(bdb[:, bi:bi + 1], bb)
        pb = psB.tile([P, 1024], F32, tag="psw")
        for ck in range(2):
            nc.tensor.matmul(pb[:, ck * 512:(ck + 1) * 512], wks[:, bi],
                             cfv[:, bi, ck * 512:(ck + 1) * 512],
                             start=True, stop=True)
        hv = hout[:, bi].rearrange("p t x -> p (t x)")
        # NB: GPSIMD cannot read PSUM on real HW — evict via Act/DVE only
        evq = [nc.scalar, nc.vector, nc.vector, nc.scalar][bi]
        if evq is nc.scalar:
            nc.scalar.activation(out=hv, in_=pb[:], func=AF.Identity,
                                 bias=bdb[:, bi:bi + 1], scale=1.0)
        else:
            evq.tensor_scalar(out=hv, in0=pb[:], scalar1=bdb[:, bi:bi + 1],
                              scalar2=0.0, op0=OP.add, op1=OP.add)

    # ========= inverse Haar (unscaled; x2 absorbed in eps) =========
    habcd = big.tile([P, 4, T, 256], F32, tag="zb")   # reuse z slab
    rec = big.tile([P, T, H, W], F32, tag="xin")      # reuse xin slab
    LLo, HLo, LHo, HHo = (hout[:, k].rearrange("p t (u w) -> p t u w", u=16)
                          for k in range(4))
    hv = habcd[:].rearrange("p k t (u w) -> p k t u w", u=16)
    hap, ham, hbp, hbm = hv[:, 0], hv[:, 1], hv[:, 2], hv[:, 3]
    # stage 1; sum(rec) = 2*(sum hap + sum ham)
    nc.vector.tensor_add(hap, LLo, HLo)
    nc.vector.tensor_sub(ham, LLo, HLo)
    nc.gpsimd.tensor_add(hbp, LHo, HHo)
    nc.gpsimd.tensor_sub(hbm, LHo, HHo)
    # stage 2 quadrants
    nc.vector.tensor_add(rec[:, :, 0::2, 0::2], hap, hbp)
    nc.vector.tensor_sub(rec[:, :, 0::2, 1::2], hap, hbp)
    nc.gpsimd.tensor_add(rec[:, :, 1::2, 0::2], ham, hbm)
    nc.gpsimd.tensor_sub(rec[:, :, 1::2, 1::2], ham, hbm)
    # sums of hap/ham and sumsq of all four intermediates
    hv4 = habcd[:].rearrange("p k t x -> p k (t x)")
    for col, kk in ((0, 0), (1, 1)):
        dv = scratch.tile([P, FT], F32, tag="dmpV3")
        nc.vector.tensor_scalar(out=dv[:], in0=hv4[:, kk], scalar1=0.0,
                                scalar2=0.0, op0=OP.add, op1=OP.add,
                                accum_out=sr[:, col:col + 1])
    for kk in range(4):
        da = scratch.tile([P, FT], F32, tag="dmpA3")
        nc.scalar.activation(out=da[:], in_=hv4[:, kk], func=AF.Square,
                             accum_out=sq[:, kk:kk + 1])
    nc.vector.tensor_reduce(out=pt3[:, 0:1], in_=sr[:], axis=AX.X, op=OP.add)
    nc.vector.tensor_scalar_mul(pt3[:, 0:1], pt3[:, 0:1], 2.0)
    nc.vector.tensor_reduce(out=pt3[:, 1:2], in_=sq[:], axis=AX.X, op=OP.add)
    nc.vector.tensor_scalar_mul(pt3[:, 1:2], pt3[:, 1:2], 2.0)

    if KSTAGE == 5:
        nc.sync.dma_start(out=out_d[:],
                          in_=rec[:].rearrange("p t h w -> p (t h w)"))
        ctx.close()
        return

    # ========= conv + inv stats -> ab3 =========
    nc.vector.tensor_reduce(out=pt4[:, 0:1], in_=sc1[:], axis=AX.X, op=OP.add)
    nc.vector.tensor_reduce(out=pt4[:, 2:3], in_=sc2[:], axis=AX.X, op=OP.add)
    nc.vector.tensor_reduce(out=pt4[:, 1:2], in_=sq1[:], axis=AX.X, op=OP.add)
    nc.vector.tensor_reduce(out=pt4[:, 3:4], in_=sq2[:], axis=AX.X, op=OP.add)
    pt34 = small.tile([P, 6], F32, tag="pt34")
    nc.vector.tensor_copy(pt34[:, 0:1], pt3[:, 0:1])
    nc.vector.tensor_copy(pt34[:, 1:2], pt4[:, 0:1])
    nc.vector.tensor_copy(pt34[:, 2:3], pt4[:, 2:3])
    nc.vector.tensor_copy(pt34[:, 3:4], pt3[:, 1:2])
    nc.vector.tensor_copy(pt34[:, 4:5], pt4[:, 1:2])
    nc.vector.tensor_copy(pt34[:, 5:6], pt4[:, 3:4])
    st3 = psS.tile([CL, 6], F32, tag="pss")
    nc.tensor.matmul(st3, selc_sb[:], pt34[:], start=True, stop=True)
    sb3 = small.tile([CL, 6], F32, tag="sb3")
    nc.vector.tensor_copy(sb3[:], st3)
    w32c = small.tile([CL, 15], F32, tag="w32c")
    _bn_small(nc, small, sb3[:, 0:3], sb3[:, 3:6], None, None,
              n=16384.0, half_s2=False, eps=bnp_sb[:, 18:21],
              g=bnp_sb[:, 12:15], b=bnp_sb[:, 15:18],
              outA=w32c[:, 0:3], outB=w32c[:, 3:6], w=w32c[:, 6:15])
    bc3 = small.tile([CL, 4], F32, tag="bc3")
    nc.vector.tensor_copy(bc3[:, 0:1], w32c[:, 0:1])
    nc.vector.tensor_reduce(out=bc3[:, 1:2], in_=w32c[:, 3:6], axis=AX.X,
                            op=OP.add)
    nc.vector.tensor_copy(bc3[:, 2:4], w32c[:, 1:3])
    bp3 = psS.tile([P, 4], F32, tag="pss")
    nc.tensor.matmul(bp3, selb_sb[:], bc3[:], start=True, stop=True)
    nc.vector.tensor_copy(ab3[:], bp3)

    # ========= final combine + store =========
    recv = rec[:].rearrange("p t h w -> p (t h w)")
    dmaq = [nc.sync, nc.scalar, nc.gpsimd, nc.sync]
    for t in range(T):
        cols = slice(t * FT, (t + 1) * FT)
        nc.scalar.activation(out=recv[:, cols], in_=recv[:, cols],
                             func=AF.Identity, bias=ab3[:, 1:2],
                             scale=ab3[:, 0:1])
        nc.vector.scalar_tensor_tensor(
            out=recv[:, cols], in0=c1[:, t], scalar=ab3[:, 2:3],
            in1=recv[:, cols], op0=OP.mult, op1=OP.add)
        pcomb = scratch.tile([P, FT], F32, tag="pcomb")
        nc.gpsimd.tensor_scalar(out=pcomb[:], in0=c2[:, t],
                                scalar1=ab3[:, 3:4], scalar2=0.0,
                                op0=OP.mult, op1=OP.add)
        nc.gpsimd.tensor_add(recv[:, cols], recv[:, cols], pcomb[:])
        dmaq[t].dma_start(out=out_d[:, t * FT:(t + 1) * FT], in_=recv[:, cols])

    ctx.close()


def _bn_small(nc, pool, S1, S2, S2a, S2b, n, half_s2, eps, g, b, outA, outB, w):
    """BN affine params on CL partitions, vectorized over k adjacent columns.

    S1: (CL,k) raw sums; S2 (or S2a+S2b when half_s2): raw sums of squares.
    outA = g * rsqrt(var + eps); outB = b - outA * mu.
    w: (CL, 3k) workspace.
    """
    k = S1.shape[1]
    nmu, ex2, t0 = w[:, 0:k], w[:, k:2 * k], w[:, 2 * k:3 * k]
    nc.vector.tensor_scalar_mul(nmu, S1, -1.0 / n)
    if half_s2:
        nc.vector.tensor_add(ex2, S2a, S2b)
        nc.vector.tensor_scalar_mul(ex2, ex2, 0.5 / n)
    else:
        nc.vector.tensor_scalar_mul(ex2, S2, 1.0 / n)
    nc.vector.tensor_mul(t0, nmu, nmu)
    nc.vector.tensor_sub(ex2, ex2, t0)                      # var
    if isinstance(eps, float):
        nc.vector.tensor_scalar_add(ex2, ex2, eps)
    else:
        nc.vector.tensor_add(ex2, ex2, eps)                 # per-column eps AP
    nc.scalar.sqrt(t0, ex2)
    nc.vector.reciprocal(t0, t0)                            # rsqrt(var+eps)
    nc.vector.tensor_mul(outA, g, t0)
    # B = b - A*mu ; nmu = -mu so B = (A * nmu) + b, done per column since
    # the STT scalar must be a single per-partition value
    for j in range(k):
        nc.vector.scalar_tensor_tensor(
            out=outB[:, j:j + 1], in0=outA[:, j:j + 1], scalar=nmu[:, j:j + 1],
            in1=b[:, j:j + 1], op0=OP.mult, op1=OP.add)


# --------------------------------------------------------------------------
# host wrapper
# --------------------------------------------------------------------------

_NC = None


def _get_module():
    global _NC
    if _NC is None:
        _NC = build_module()
    return _NC


def _host_prep(inputs):
    """Build the 8 per-core input maps from full inputs."""
    x = np.asarray(inputs["x"], np.float32)
    haar_weight = np.asarray(inputs["haar_weight"], np.float32)
    conv1_w = np.asarray(inputs["conv1_w"], np.float32)
    conv2_w = np.asarray(inputs["conv2_w"], np.float32)

    # block-diag selector matrices (shared)
    selc = np.zeros((P, CL), np.float32)
    selc[np.arange(P), np.arange(P) % CL] = 1.0
    selb = np.ascontiguousarray(selc.T)

    # conv stationaries: lhsT[(b,i), (b,o)] = W[o,i] within each 16-group
    def blockdiag16(w_oi):  # (16,16) -> (128,128) lhsT
        m = np.zeros((P, P), np.float32)
        for g in range(8):      # 4 b * 2 groups
            m[g * 16:(g + 1) * 16, g * 16:(g + 1) * 16] = w_oi.T
        return m

    w1blk = _to_bf16(blockdiag16(conv1_w[:, :, 0, 0]))
    w2blk = _to_bf16(np.stack([blockdiag16(conv2_w[:, :, dy, dx])
                               for dy in range(3) for dx in range(3)]))

    in_maps = []
    for dd in range(NCORES):
        c0 = CL * dd
        sl = slice(c0, c0 + CL)
        x_core = np.ascontiguousarray(
            x[:, :, sl].transpose(1, 2, 0, 3, 4)).reshape(P, F)
        # block-diag stationaries: lhsT[(b,g,d),(b,g,m)] = Wk[d,m]
        wkblk = np.zeros((4, P, P), np.float32)
        for k in range(4):
            wk = haar_weight[4 * k + dd // 2]
            for g in range(8):
                wkblk[k, g * 16:(g + 1) * 16, g * 16:(g + 1) * 16] = wk
        wk_host = np.ascontiguousarray(wkblk.transpose(1, 0, 2)).reshape(P, 4 * P)
        w2_host = np.ascontiguousarray(w2blk.transpose(1, 0, 2)).reshape(P, 9 * P)

        bnp = np.zeros((CL, 21), np.float32)
        bnp[:, 0] = inputs["bn_fwd_g"][sl]
        bnp[:, 1] = inputs["bn_fwd_g"][C + c0:C + c0 + CL]
        bnp[:, 2] = inputs["bn_fwd_b"][sl]
        bnp[:, 3] = inputs["bn_fwd_b"][C + c0:C + c0 + CL]
        gm = np.asarray(inputs["bn_mul_g"], np.float32).reshape(4, C)[:, sl]
        bm = np.asarray(inputs["bn_mul_b"], np.float32).reshape(4, C)[:, sl]
        bnp[:, 4:8] = gm.T
        bnp[:, 8:12] = bm.T
        bnp[:, 12] = inputs["bn_inv_g"][sl]
        bnp[:, 13] = inputs["bn_c1_g"][sl]
        bnp[:, 14] = inputs["bn_c2_g"][sl]
        bnp[:, 15] = inputs["bn_inv_b"][sl]
        bnp[:, 16] = inputs["bn_c1_b"][sl]
        bnp[:, 17] = inputs["bn_c2_b"][sl]
        bnp[:, 18] = 4e-5   # bn_inv eps (x4: unscaled inverse haar)
        bnp[:, 19] = 1e-5   # bn_c1 eps
        bnp[:, 20] = 1e-5   # bn_c2 eps

        in_maps.append({
            "xin": x_core,
            "w1blk": w1blk,
            "w2blk": w2_host,
            "wkblk": wk_host,
            "selc": selc,
            "selb": selb,
            "bnp": np.ascontiguousarray(bnp),
        })
    return in_maps


def _to_bf16(a):
    return np.asarray(a, dtype=mybir.dt.np(BF16))


def _assemble(results):
    out = np.zeros((T, B, C, H, W), np.float32)
    for dd in range(NCORES):
        oc = np.asarray(results[dd]["out"]).reshape(B, CL, T, H, W)
        out[:, :, CL * dd:CL * (dd + 1)] = oc.transpose(2, 0, 1, 3, 4)
    return out


def kernel(**inputs):
    nc = _get_module()
    in_maps = _host_prep(inputs)
    res = run_bass_kernel_spmd(nc, in_maps, list(range(NCORES)))
    return _assemble(res.results)


if __name__ == "__main__":
    # smoke test with random inputs
    rng = np.random.default_rng(0)
    inputs = {
        "x": rng.standard_normal((T, B, C, H, W)).astype(np.float32),
        "haar_weight": 0.02 * rng.standard_normal((16, 16, 16)).astype(np.float32),
        "conv1_w": 0.1 * rng.standard_normal((16, 16, 1, 1)).astype(np.float32),
        "conv1_b": np.zeros(16, np.float32),
        "conv2_w": 0.05 * rng.standard_normal((16, 16, 3, 3)).astype(np.float32),
        "conv2_b": np.zeros(16, np.float32),
        "bn_fwd_g": np.ones(512, np.float32), "bn_fwd_b": np.zeros(512, np.float32),
        "bn_mul_g": np.ones(1024, np.float32), "bn_mul_b": np.zeros(1024, np.float32),
        "bn_inv_g": np.ones(256, np.float32), "bn_inv_b": np.zeros(256, np.float32),
        "bn_c1_g": np.ones(256, np.float32), "bn_c1_b": np.zeros(256, np.float32),
        "bn_c2_g": np.ones(256, np.float32), "bn_c2_b": np.zeros(256, np.float32),
    }
    out = kernel(**inputs)
    print("out", out.shape, out.dtype, np.abs(out).mean())


# revision 33
# speedup vs baseline: 1.4984x; 1.0812x over previous
"""Trainium2 Bass kernel for nn_FATMSparse (spiking Haar-wavelet network).

Sharding: the 256 channels are split 32-per-core across 8 cores. Every
stage of the network (LIF, Haar transforms, all five training-mode
BatchNorms, the per-16-channel block-diagonal mixes and both grouped
convolutions) is exactly local to an aligned 32-channel slice, so there
are no cross-core collectives at all and BN statistics are exact.

Per-core layout: SBUF partitions p = b*32 + c_local (128), free = (t,h,w).

v2 notes (vs the 93us baseline):
 - convs run on single-precision bf16 weights (one matmul per tap);
 - spikes are written into a halo-padded bf16 tile so conv taps and the
   Haar transform read it directly (no separate padded copy);
 - LIF uses the doubled-potential recurrence w=2v (3 ops/step);
 - all BN statistics ride accum_out side-outputs (Act Square+accum, DVE
   tensor_tensor_reduce, tensor_scalar accum) instead of square+reduce
   passes; level-1 Haar stats use the spike-integer identities
   sum(ulo^2)=sum(s)+2*cnt{ulo=2}, sum(uhi^2)=sum(s)-2*cnt{ulo=2};
 - conv bias is dropped entirely (training-mode BN cancels it).

Self-contained: hardcodes all shapes; imports concourse from /opt/trn_rl_repo.
"""
import os
import sys

sys.path.insert(0, "/opt/trn_rl_repo")

import numpy as np

import concourse.bass as bass
import concourse.bacc as bacc
import concourse.tile as tile
from concourse import mybir
from concourse.bass_utils import run_bass_kernel_spmd

F32 = mybir.dt.float32
F32R = mybir.dt.float32r
BF16 = mybir.dt.bfloat16
AX = mybir.AxisListType
OP = mybir.AluOpType
AF = mybir.ActivationFunctionType

T, B, C, H, W = 4, 4, 256, 32, 32
CL = 32               # channels per core
NCORES = 8
P = 128               # partitions = B * CL
FT = H * W            # 1024 free per t
F = T * FT            # 4096
INV_SQRT2 = float(np.float32(1.0 / np.sqrt(2.0)))
SQRT2B = float(np.float32(2.0) * np.float32(INV_SQRT2))   # scale for B' fold
TAUS = [0.01, 0.02, 0.02, 0.05]
RD = 19               # LIF/DVE takes h-rows [0, RD), Pool takes [RD, H)


def build_module():
    nc = bacc.Bacc("TRN2", target_bir_lowering=False, debug=False)

    def din(name, shape, dt):
        return nc.dram_tensor(name, shape, dt, kind="ExternalInput").ap()

    xin_d = din("xin", [P, F], F32)
    w1_d = din("w1blk", [P, P], BF16)
    w2_d = din("w2blk", [P, 9 * P], BF16)
    wk_d = din("wkblk", [P, 4 * P], F32)
    selc_d = din("selc", [P, CL], F32)
    selb_d = din("selb", [CL, P], F32)
    bnp_d = din("bnp", [CL, 21], F32)
    out_d = nc.dram_tensor("out", [P, F], F32, kind="ExternalOutput").ap()

    with tile.TileContext(nc) as tc:
        _emit(tc, nc, xin_d, w1_d, w2_d, wk_d, selc_d, selb_d, bnp_d, out_d)
    nc.finalize()
    return nc


def _emit(tc, nc, xin_d, w1_d, w2_d, wk_d, selc_d, selb_d, bnp_d, out_d):
    import contextlib

    ctx = contextlib.ExitStack()
    consts = ctx.enter_context(tc.tile_pool(name="consts", bufs=1))
    big = ctx.enter_context(tc.tile_pool(name="big", bufs=1))
    scratch = ctx.enter_context(tc.tile_pool(name="scratch", bufs=1))
    small = ctx.enter_context(tc.tile_pool(name="small", bufs=1))
    psB = ctx.enter_context(tc.tile_pool(name="psB", bufs=3, space="PSUM"))
    psS = ctx.enter_context(tc.tile_pool(name="psS", bufs=2, space="PSUM"))

    # ---- constant loads (gpsimd queue) ----
    w1_sb = consts.tile([P, P], BF16, tag="w1")
    nc.gpsimd.dma_start(out=w1_sb, in_=w1_d[:])
    w2_sb = consts.tile([P, 9, P], BF16, tag="w2")
    nc.gpsimd.dma_start(out=w2_sb, in_=w2_d[:].rearrange("p (k n) -> p k n", k=9))
    wk_sb = consts.tile([P, 4, P], F32, tag="wk")
    nc.gpsimd.dma_start(out=wk_sb, in_=wk_d[:].rearrange("p (k n) -> p k n", k=4))
    selc_sb = consts.tile([P, CL], F32, tag="selc")
    nc.gpsimd.dma_start(out=selc_sb, in_=selc_d[:])
    selb_sb = consts.tile([CL, P], F32, tag="selb")
    nc.gpsimd.dma_start(out=selb_sb, in_=selb_d[:])
    bnp_sb = consts.tile([CL, 21], F32, tag="bnp")
    nc.gpsimd.dma_start(out=bnp_sb, in_=bnp_d[:])

    # ---- big tiles ----
    xin = big.tile([P, T, FT], F32, tag="xin")
    wv = big.tile([P, H, W], F32, tag="wv")        # 2*v LIF state
    mv = big.tile([P, H, W], F32, tag="mv")        # masked w
    s = big.tile([P, T, H + 2, W + 2], BF16, tag="s")
    ulo = big.tile([P, T, H, 16], BF16, tag="ulo")
    uhi = big.tile([P, T, H, 16], BF16, tag="uhi")
    plo = big.tile([P, T, 16, 16], BF16, tag="plo")
    qlo = big.tile([P, T, 16, 16], BF16, tag="qlo")
    phi = big.tile([P, T, 16, 16], BF16, tag="phi")
    qhi = big.tile([P, T, 16, 16], BF16, tag="qhi")
    zb = big.tile([P, 4, T, 256], F32, tag="zb")   # z per band
    cf = big.tile([P, 4, T, 256], F32R, tag="cf")  # gated coeffs / matmul rhs
    hout = big.tile([P, 4, T, 256], F32, tag="hout")
    c1 = big.tile([P, T, FT], F32, tag="c1")
    c2 = big.tile([P, T, FT], F32, tag="c2")

    # ---- small stats tiles ----
    su_t = small.tile([P, 4], F32, tag="su_t")     # sum(ulo) per t
    cn_t = small.tile([P, 4], F32, tag="cn_t")     # cnt{ulo==2} per t
    sh_t = small.tile([P, 4], F32, tag="sh_t")     # sum(uhi) per t
    pt1 = small.tile([P, 3], F32, tag="pt1")
    sc1 = small.tile([P, 4], F32, tag="sc1")       # conv sums per t
    sq1 = small.tile([P, 4], F32, tag="sq1")
    sc2 = small.tile([P, 4], F32, tag="sc2")
    sq2 = small.tile([P, 4], F32, tag="sq2")
    ett = small.tile([P, 4, 4], F32, tag="ett")    # band energy per (band, t)
    mek = small.tile([P, 4, 4], F32, tag="mek")
    s1acc = small.tile([P, 4, 4], F32, tag="s1acc")
    pt2 = small.tile([P, 8], F32, tag="pt2")
    sr = small.tile([P, 2], F32, tag="sr")         # sum(hap), sum(ham)
    sq = small.tile([P, 4], F32, tag="sq")         # sumsq of hap/ham/hbp/hbm
    pt3 = small.tile([P, 2], F32, tag="pt3")
    pt4 = small.tile([P, 4], F32, tag="pt4")
    ab1 = small.tile([P, 4], F32, tag="ab1")       # A'lo A'hi B'lo B'hi
    ab2 = small.tile([P, 8], F32, tag="ab2")
    ab3 = small.tile([P, 4], F32, tag="ab3")       # A_r Btot A1 A2c
    wks = small.tile([P, 4, P], F32R, tag="wks")
    bdb = small.tile([P, 4], F32, tag="bdb")

    xv = xin[:].rearrange("p t (h w) -> p t h w", w=W)
    KSTAGE = int(os.environ.get("KSTAGE", "9"))

    # force the sqrt_and_others act table once, up front (covers Copy/
    # Identity/Square/Sqrt) so no mid-stream table reload occurs
    dsq = small.tile([P, 1], F32, tag="dsq")
    nc.vector.memset(dsq[:], 1.0)
    nc.scalar.sqrt(dsq[:], dsq[:])

    # halo borders of s zeroed once (rows 0 and H+1, cols 0 and W+1)
    nc.gpsimd.memset(s[:, :, 0, :], 0.0)
    nc.gpsimd.memset(s[:, :, H + 1, :], 0.0)
    nc.gpsimd.memset(s[:, :, 1:H + 1, 0], 0.0)
    nc.gpsimd.memset(s[:, :, 1:H + 1, W + 1], 0.0)

    # input DMAs: keep the Act queue nearly free (SP carries 7 of 8)
    nc.sync.dma_start(out=xin[:, 0, 0:512], in_=xin_d[:, 0:512])
    nc.scalar.dma_start(out=xin[:, 0, 512:FT], in_=xin_d[:, 512:FT])
    for t in range(1, T):
        nc.sync.dma_start(out=xin[:, t, 0:512], in_=xin_d[:, t * FT:t * FT + 512])
        nc.sync.dma_start(out=xin[:, t, 512:FT],
                          in_=xin_d[:, t * FT + 512:(t + 1) * FT])

    lif_split = [(nc.vector, 0, RD), (nc.gpsimd, RD, H)]

    def conv_t(t, evict2):
        """PE matmuls + eviction/stat side-ops for time step t."""
        p1 = psB.tile([P, 1024], F32, tag="psw")
        p2 = psB.tile([P, 1024], F32, tag="psw")
        for ck in range(2):
            hs = ck * 16
            nc.tensor.matmul(p1[:, ck * 512:(ck + 1) * 512], w1_sb[:],
                             s[:, t, 1 + hs:17 + hs, 1:W + 1],
                             start=True, stop=True)
        for ck in range(2):
            hs = ck * 16
            for i, (dy, dx) in enumerate([(a, b) for a in range(3)
                                          for b in range(3)]):
                nc.tensor.matmul(p2[:, ck * 512:(ck + 1) * 512], w2_sb[:, i],
                                 s[:, t, hs + dy:hs + dy + 16, dx:dx + 32],
                                 start=(i == 0), stop=(i == 8))
        # evictions with fused sums; sumsq straight off PSUM
        nc.scalar.activation(out=c1[:, t], in_=p1[:], func=AF.Copy,
                             accum_out=sc1[:, t:t + 1])
        dmpA = scratch.tile([P, 1024], F32, tag="dmpA")
        nc.scalar.activation(out=dmpA[:], in_=p1[:], func=AF.Square,
                             accum_out=sq1[:, t:t + 1])
        dmpA2 = scratch.tile([P, 1024], F32, tag="dmpA")
        nc.scalar.activation(out=dmpA2[:], in_=p2[:], func=AF.Square,
                             accum_out=sq2[:, t:t + 1])
        if evict2 is nc.scalar:
            nc.scalar.activation(out=c2[:, t], in_=p2[:], func=AF.Copy,
                                 accum_out=sc2[:, t:t + 1])
        else:
            evict2.tensor_scalar(out=c2[:, t], in0=p2[:], scalar1=0.0,
                                 scalar2=0.0, op0=OP.add, op1=OP.add,
                                 accum_out=sc2[:, t:t + 1])

    # ========= phase A+B+C: LIF, Haar W/H, fwd stats (DVE/Pool) =========
    for t in range(T):
        for eng, r0, r1 in lif_split:
            dve = eng is nc.vector
            xt = xv[:, t, r0:r1]
            wt, mt = wv[:, r0:r1], mv[:, r0:r1]
            st = s[:, t, 1 + r0:1 + r1, 1:W + 1]
            if t == 0:
                eng.tensor_single_scalar(out=st, in_=xt, scalar=2.0, op=OP.is_ge)
                if dve:
                    eng.scalar_tensor_tensor(out=mt, in0=xt, scalar=2.0,
                                             in1=xt, op0=OP.is_lt, op1=OP.mult)
                else:
                    # Pool has no scalar_tensor_tensor on real HW
                    eng.tensor_single_scalar(out=mt, in_=xt, scalar=2.0,
                                             op=OP.is_lt)
                    eng.tensor_mul(mt, xt, mt)
            else:
                if dve:
                    eng.scalar_tensor_tensor(out=wt, in0=mt, scalar=0.5,
                                             in1=xt, op0=OP.mult, op1=OP.add)
                else:
                    eng.tensor_scalar_mul(mt, mt, 0.5)
                    eng.tensor_add(wt, mt, xt)
                eng.tensor_single_scalar(out=st, in_=wt, scalar=2.0, op=OP.is_ge)
                if t < T - 1:
                    if dve:
                        eng.scalar_tensor_tensor(out=mt, in0=wt, scalar=2.0,
                                                 in1=wt, op0=OP.is_lt,
                                                 op1=OP.mult)
                    else:
                        eng.tensor_single_scalar(out=mt, in_=wt, scalar=2.0,
                                                 op=OP.is_lt)
                        eng.tensor_mul(mt, wt, mt)
        # Haar along W (unscaled, bf16-exact): ulo=se+so, uhi=se-so
        se = s[:, t, 1:H + 1, 1:W + 1:2]
        so = s[:, t, 1:H + 1, 2:W + 2:2]
        nc.gpsimd.tensor_add(ulo[:, t], se, so)
        nc.gpsimd.tensor_sub(uhi[:, t], se, so)
        # Haar along H: plo/qlo on DVE (bf16 2x), phi/qhi on Pool
        ue, uo = ulo[:, t, 0::2, :], ulo[:, t, 1::2, :]
        he, ho = uhi[:, t, 0::2, :], uhi[:, t, 1::2, :]
        nc.vector.tensor_add(plo[:, t], ue, uo)
        nc.vector.tensor_sub(qlo[:, t], ue, uo)
        nc.gpsimd.tensor_add(phi[:, t], he, ho)
        nc.gpsimd.tensor_sub(qhi[:, t], he, ho)

    # fwd stats (single full-tensor bf16 accums; spike-integer identities):
    # sum(l1_lo raw) = sum(ulo) = sum(s); cnt2 = #{ulo==2};
    # sum(ulo^2) = sum(s)+2*cnt2 ; sum(uhi^2) = sum(s)-2*cnt2
    ulof = ulo[:].rearrange("p t h w -> p (t h w)")
    uhif = uhi[:].rearrange("p t h w -> p (t h w)")
    dv = scratch.tile([P, 2048], BF16, tag="dmpV")
    nc.vector.tensor_scalar(out=dv[:], in0=ulof, scalar1=0.0,
                            scalar2=0.0, op0=OP.add, op1=OP.add,
                            accum_out=pt1[:, 0:1])
    dv2 = scratch.tile([P, 2048], BF16, tag="dmpV")
    nc.vector.tensor_scalar(out=dv2[:], in0=uhif, scalar1=0.0,
                            scalar2=0.0, op0=OP.add, op1=OP.add,
                            accum_out=pt1[:, 1:2])
    dv3 = scratch.tile([P, 2048], BF16, tag="dmpV")
    nc.vector.tensor_scalar(out=dv3[:], in0=ulof, scalar1=2.0,
                            scalar2=0.0, op0=OP.is_ge, op1=OP.add,
                            accum_out=pt1[:, 2:3])

    if KSTAGE == 1:
        nc.sync.dma_start(out=out_d[:, 0:FT],
                          in_=wv[:].rearrange("p h w -> p (h w)"))
        ctx.close()
        return

    # ========= convs t0/t1, then fwd BN params, then convs t2/t3 =========
    # Ordering puts st1/bp1 into the PE queue between conv-t1 and conv-t2 so
    # the band phase can start while conv-t2/t3 still run; late c2 evictions
    # go to Act so DVE/Pool are free for the band phase.
    conv_t(0, nc.scalar)
    conv_t(1, nc.scalar)
    st1 = psS.tile([CL, 3], F32, tag="pss")
    nc.tensor.matmul(st1, selc_sb[:], pt1[:], start=True, stop=True)
    stc = small.tile([CL, 3], F32, tag="stc")
    nc.vector.tensor_copy(stc[:], st1)
    sb1 = small.tile([CL, 4], F32, tag="sb1")
    # [S1lo, S1hi, S2lo, S2hi]; S2lo = S + 2*cnt2, S2hi = S - 2*cnt2
    nc.vector.tensor_copy(sb1[:, 0:2], stc[:, 0:2])
    nc.vector.scalar_tensor_tensor(out=sb1[:, 2:3], in0=stc[:, 2:3], scalar=2.0,
                                   in1=stc[:, 0:1], op0=OP.mult, op1=OP.add)
    nc.vector.scalar_tensor_tensor(out=sb1[:, 3:4], in0=stc[:, 2:3], scalar=-2.0,
                                   in1=stc[:, 0:1], op0=OP.mult, op1=OP.add)
    w32 = small.tile([CL, 10], F32, tag="w32")
    _bn_small(nc, small, sb1[:, 0:2], sb1[:, 2:4], None, None,
              n=8192.0, half_s2=False, eps=2e-5,
              g=bnp_sb[:, 0:2], b=bnp_sb[:, 2:4],
              outA=w32[:, 0:2], outB=w32[:, 2:4], w=w32[:, 4:10])
    bc1 = small.tile([CL, 4], F32, tag="bc1")
    nc.vector.tensor_scalar_mul(bc1[:, 0:2], w32[:, 0:2], INV_SQRT2)   # A'
    nc.vector.tensor_scalar_mul(bc1[:, 2:4], w32[:, 2:4], SQRT2B)      # B'
    bp1 = psS.tile([P, 4], F32, tag="pss")
    nc.tensor.matmul(bp1, selb_sb[:], bc1[:], start=True, stop=True)
    nc.vector.tensor_copy(ab1[:], bp1)
    conv_t(2, nc.scalar)
    conv_t(3, nc.scalar)

    if KSTAGE == 3:
        nc.sync.dma_start(out=out_d[:, 0:FT],
                          in_=plo[:].rearrange("p t u w -> p (t u w)"))
        nc.sync.dma_start(out=out_d[:, FT:FT + 4], in_=ab1[:])
        ctx.close()
        return

    # ========= bands: z, gate, energy, cf =========
    # band order: LL(plo,+B), HL(qlo), LH(phi,+B), HH(qhi)
    band_src = [(plo, 0, True), (qlo, 0, False), (phi, 1, True), (qhi, 1, False)]
    # z+m on DVE for all bands; u on pool for LH/HH, DVE for LL/HL;
    # E via DVE TTR (LL/HL) and Act Square+accum (LH/HH)
    for bi, (pq, ci, has_b) in enumerate(band_src):
        pqv = pq[:].rearrange("p t u w -> p t (u w)")
        a_ap = ab1[:, ci:ci + 1]
        z = zb[:, bi]
        zf = z.rearrange("p t x -> p (t x)")
        on_dve = bi < 2
        ev, ep = (nc.vector, nc.gpsimd)
        if has_b:
            (ev if on_dve else ep).tensor_scalar(
                out=z, in0=pqv, scalar1=a_ap, scalar2=ab1[:, 2 + ci:3 + ci],
                op0=OP.mult, op1=OP.add)
        else:
            (ev if on_dve else ep).tensor_scalar(
                out=z, in0=pqv, scalar1=a_ap, scalar2=0.0,
                op0=OP.mult, op1=OP.add)
        zz = scratch.tile([P, T, 256], F32, tag="mband" + str(bi))
        (ev if on_dve else ep).tensor_mul(zz[:], z, z)
        u = cf[:, bi]
        if on_dve:
            nc.vector.scalar_tensor_tensor(out=u, in0=zz[:], scalar=0.25,
                                           in1=z, op0=OP.is_ge, op1=OP.mult)
            # E(t) = sum(gate*zz) = sum(u^2), fused accum on the DVE STT
            for t in range(T):
                dv4 = scratch.tile([P, 256], F32, tag="dmpV2")
                nc.vector.scalar_tensor_tensor(
                    out=dv4[:], in0=zz[:, t], scalar=0.25, in1=zz[:, t],
                    op0=OP.is_ge, op1=OP.mult,
                    accum_out=ett[:, bi, t:t + 1])
        else:
            mg = scratch.tile([P, T, 256], F32, tag="mgband")
            ep.tensor_single_scalar(out=mg[:], in_=zz[:], scalar=0.25,
                                    op=OP.is_ge)
            ep.tensor_mul(u, mg[:], z)
            for t in range(T):
                da = scratch.tile([P, 256], F32, tag="dmpA2")
                nc.scalar.activation(out=da[:], in_=u[:, t], func=AF.Square,
                                     accum_out=ett[:, bi, t:t + 1])
        thr = float(np.float32(256.0) * np.float32(TAUS[bi]))
        (ev if on_dve else ep).tensor_single_scalar(
            out=mek[:, bi], in_=ett[:, bi], scalar=thr, op=OP.is_gt)
        for t in range(T):
            nc.vector.tensor_scalar(
                out=u[:, t], in0=u[:, t], scalar1=mek[:, bi, t:t + 1],
                scalar2=0.0, op0=OP.mult, op1=OP.add,
                accum_out=s1acc[:, bi, t:t + 1])

    # BN_mul stats -> ab2
    for bi in range(4):
        nc.vector.tensor_reduce(out=pt2[:, bi:bi + 1], in_=s1acc[:, bi],
                                axis=AX.X, op=OP.add)
    nc.vector.tensor_mul(mek[:], mek[:], ett[:])     # maskE * E  (in place)
    for bi in range(4):
        nc.vector.tensor_reduce(out=pt2[:, 4 + bi:5 + bi], in_=mek[:, bi],
                                axis=AX.X, op=OP.add)
    st2 = psS.tile([CL, 8], F32, tag="pss")
    nc.tensor.matmul(st2, selc_sb[:], pt2[:], start=True, stop=True)
    sb2 = small.tile([CL, 8], F32, tag="sb2")
    nc.vector.tensor_copy(sb2[:], st2)
    w32b = small.tile([CL, 20], F32, tag="w32b")
    _bn_small(nc, small, sb2[:, 0:4], sb2[:, 4:8], None, None,
              n=4096.0, half_s2=False, eps=1e-5,
              g=bnp_sb[:, 4:8], b=bnp_sb[:, 8:12],
              outA=w32b[:, 0:4], outB=w32b[:, 4:8], w=w32b[:, 8:20])
    bp2 = psS.tile([P, 8], F32, tag="pss")
    nc.tensor.matmul(bp2, selb_sb[:], w32b[:, 0:8], start=True, stop=True)
    nc.vector.tensor_copy(ab2[:], bp2)

    if KSTAGE == 4:
        nc.sync.dma_start(out=out_d[:],
                          in_=cf[:].rearrange("p k t x -> p (k t x)"))
        ctx.close()
        return

    # ========= block-diagonal multiply (BN_mul folded in) =========
    cfv = cf[:].rearrange("p k t x -> p k (t x)")
    for bi in range(4):
        nc.vector.tensor_scalar_mul(wks[:, bi], wk_sb[:, bi], ab2[:, bi:bi + 1])
        bb = psS.tile([P, 1], F32, tag="pss")
        nc.tensor.matmul(bb, wk_sb[:, bi], ab2[:, 4 + bi:5 + bi],
                         start=True, stop=True)
        nc.vector.tensor_copy(bdb[:, bi:bi + 1], bb)
        pb = psB.tile([P, 1024], F32, tag="psw")
        for ck in range(2):
            nc.tensor.matmul(pb[:, ck * 512:(ck + 1) * 512], wks[:, bi],
                             cfv[:, bi, ck * 512:(ck + 1) * 512],
                             start=True, stop=True)
        hv = hout[:, bi].rearrange("p t x -> p (t x)")
        # NB: GPSIMD cannot read PSUM on real HW — evict via Act/DVE only
        evq = [nc.scalar, nc.vector, nc.vector, nc.scalar][bi]
        if evq is nc.scalar:
            nc.scalar.activation(out=hv, in_=pb[:], func=AF.Identity,
                                 bias=bdb[:, bi:bi + 1], scale=1.0)
        else:
            evq.tensor_scalar(out=hv, in0=pb[:], scalar1=bdb[:, bi:bi + 1],
                              scalar2=0.0, op0=OP.add, op1=OP.add)

    # ========= inverse Haar (unscaled; x2 absorbed in eps) =========
    habcd = big.tile([P, 4, T, 256], F32, tag="zb")   # reuse z slab
    rec = big.tile([P, T, H, W], F32, tag="xin")      # reuse xin slab
    LLo, HLo, LHo, HHo = (hout[:, k].rearrange("p t (u w) -> p t u w", u=16)
                          for k in range(4))
    hv = habcd[:].rearrange("p k t (u w) -> p k t u w", u=16)
    hap, ham, hbp, hbm = hv[:, 0], hv[:, 1], hv[:, 2], hv[:, 3]
    # stage 1; sum(rec) = 2*(sum hap + sum ham)
    nc.vector.tensor_add(hap, LLo, HLo)
    nc.vector.tensor_sub(ham, LLo, HLo)
    nc.gpsimd.tensor_add(hbp, LHo, HHo)
    nc.gpsimd.tensor_sub(hbm, LHo, HHo)
    # stage 2 quadrants
    nc.vector.tensor_add(rec[:, :, 0::2, 0::2], hap, hbp)
    nc.vector.tensor_sub(rec[:, :, 0::2, 1::2], hap, hbp)
    nc.gpsimd.tensor_add(rec[:, :, 1::2, 0::2], ham, hbm)
    nc.gpsimd.tensor_sub(rec[:, :, 1::2, 1::2], ham, hbm)
    # sums of hap/ham and sumsq of all four intermediates
    hv4 = habcd[:].rearrange("p k t x -> p k (t x)")
    for col, kk in ((0, 0), (1, 1)):
        dv = scratch.tile([P, FT], F32, tag="dmpV3")
        nc.vector.tensor_scalar(out=dv[:], in0=hv4[:, kk], scalar1=0.0,
                                scalar2=0.0, op0=OP.add, op1=OP.add,
                                accum_out=sr[:, col:col + 1])
    for kk in range(2):
        da = scratch.tile([P, FT], F32, tag="dmpA3")
        nc.scalar.activation(out=da[:], in_=hv4[:, kk], func=AF.Square,
                             accum_out=sq[:, kk:kk + 1])
    for kk in range(2, 4):
        dp2 = scratch.tile([P, FT], F32, tag="dmpP3")
        nc.gpsimd.tensor_mul(dp2[:], hv4[:, kk], hv4[:, kk])
        dv5 = scratch.tile([P, FT], F32, tag="dmpV3")
        nc.vector.tensor_scalar(out=dv5[:], in0=dp2[:], scalar1=0.0,
                                scalar2=0.0, op0=OP.add, op1=OP.add,
                                accum_out=sq[:, kk:kk + 1])
    nc.vector.tensor_reduce(out=pt3[:, 0:1], in_=sr[:], axis=AX.X, op=OP.add)
    nc.vector.tensor_scalar_mul(pt3[:, 0:1], pt3[:, 0:1], 2.0)
    nc.vector.tensor_reduce(out=pt3[:, 1:2], in_=sq[:], axis=AX.X, op=OP.add)
    nc.vector.tensor_scalar_mul(pt3[:, 1:2], pt3[:, 1:2], 2.0)

    if KSTAGE == 5:
        nc.sync.dma_start(out=out_d[:],
                          in_=rec[:].rearrange("p t h w -> p (t h w)"))
        ctx.close()
        return

    # ========= conv + inv stats -> ab3 =========
    nc.vector.tensor_reduce(out=pt4[:, 0:1], in_=sc1[:], axis=AX.X, op=OP.add)
    nc.vector.tensor_reduce(out=pt4[:, 2:3], in_=sc2[:], axis=AX.X, op=OP.add)
    nc.vector.tensor_reduce(out=pt4[:, 1:2], in_=sq1[:], axis=AX.X, op=OP.add)
    nc.vector.tensor_reduce(out=pt4[:, 3:4], in_=sq2[:], axis=AX.X, op=OP.add)
    pt34 = small.tile([P, 6], F32, tag="pt34")
    nc.vector.tensor_copy(pt34[:, 0:1], pt3[:, 0:1])
    nc.vector.tensor_copy(pt34[:, 1:2], pt4[:, 0:1])
    nc.vector.tensor_copy(pt34[:, 2:3], pt4[:, 2:3])
    nc.vector.tensor_copy(pt34[:, 3:4], pt3[:, 1:2])
    nc.vector.tensor_copy(pt34[:, 4:5], pt4[:, 1:2])
    nc.vector.tensor_copy(pt34[:, 5:6], pt4[:, 3:4])
    st3 = psS.tile([CL, 6], F32, tag="pss")
    nc.tensor.matmul(st3, selc_sb[:], pt34[:], start=True, stop=True)
    sb3 = small.tile([CL, 6], F32, tag="sb3")
    nc.vector.tensor_copy(sb3[:], st3)
    w32c = small.tile([CL, 15], F32, tag="w32c")
    _bn_small(nc, small, sb3[:, 0:3], sb3[:, 3:6], None, None,
              n=16384.0, half_s2=False, eps=bnp_sb[:, 18:21],
              g=bnp_sb[:, 12:15], b=bnp_sb[:, 15:18],
              outA=w32c[:, 0:3], outB=w32c[:, 3:6], w=w32c[:, 6:15])
    bc3 = small.tile([CL, 4], F32, tag="bc3")
    nc.vector.tensor_copy(bc3[:, 0:1], w32c[:, 0:1])
    nc.vector.tensor_reduce(out=bc3[:, 1:2], in_=w32c[:, 3:6], axis=AX.X,
                            op=OP.add)
    nc.vector.tensor_copy(bc3[:, 2:4], w32c[:, 1:3])
    bp3 = psS.tile([P, 4], F32, tag="pss")
    nc.tensor.matmul(bp3, selb_sb[:], bc3[:], start=True, stop=True)
    nc.vector.tensor_copy(ab3[:], bp3)

    # ========= final combine + store =========
    recv = rec[:].rearrange("p t h w -> p (t h w)")
    dmaq = [nc.sync, nc.scalar, nc.gpsimd, nc.sync]
    for t in range(T):
        cols = slice(t * FT, (t + 1) * FT)
        if t < 2:
            nc.vector.tensor_scalar(out=recv[:, cols], in0=recv[:, cols],
                                    scalar1=ab3[:, 0:1], scalar2=ab3[:, 1:2],
                                    op0=OP.mult, op1=OP.add)
        else:
            nc.scalar.activation(out=recv[:, cols], in_=recv[:, cols],
                                 func=AF.Identity, bias=ab3[:, 1:2],
                                 scale=ab3[:, 0:1])
        nc.vector.scalar_tensor_tensor(
            out=recv[:, cols], in0=c1[:, t], scalar=ab3[:, 2:3],
            in1=recv[:, cols], op0=OP.mult, op1=OP.add)
        pcomb = scratch.tile([P, FT], F32, tag="pcomb")
        nc.gpsimd.tensor_scalar(out=pcomb[:], in0=c2[:, t],
                                scalar1=ab3[:, 3:4], scalar2=0.0,
                                op0=OP.mult, op1=OP.add)
        nc.gpsimd.tensor_add(recv[:, cols], recv[:, cols], pcomb[:])
        dmaq[t].dma_start(out=out_d[:, t * FT:(t + 1) * FT], in_=recv[:, cols])

    ctx.close()


def _bn_small(nc, pool, S1, S2, S2a, S2b, n, half_s2, eps, g, b, outA, outB, w):
    """BN affine params on CL partitions, vectorized over k adjacent columns.

    S1: (CL,k) raw sums; S2 (or S2a+S2b when half_s2): raw sums of squares.
    outA = g * rsqrt(var + eps); outB = b - outA * mu.
    w: (CL, 3k) workspace.
    """
    k = S1.shape[1]
    nmu, ex2, t0 = w[:, 0:k], w[:, k:2 * k], w[:, 2 * k:3 * k]
    nc.vector.tensor_scalar_mul(nmu, S1, -1.0 / n)
    if half_s2:
        nc.vector.tensor_add(ex2, S2a, S2b)
        nc.vector.tensor_scalar_mul(ex2, ex2, 0.5 / n)
    else:
        nc.vector.tensor_scalar_mul(ex2, S2, 1.0 / n)
    nc.vector.tensor_mul(t0, nmu, nmu)
    nc.vector.tensor_sub(ex2, ex2, t0)                      # var
    if isinstance(eps, float):
        nc.vector.tensor_scalar_add(ex2, ex2, eps)
    else:
        nc.vector.tensor_add(ex2, ex2, eps)                 # per-column eps AP
    nc.scalar.sqrt(t0, ex2)
    nc.vector.reciprocal(t0, t0)                            # rsqrt(var+eps)
    nc.vector.tensor_mul(outA, g, t0)
    # B = b - A*mu ; nmu = -mu so B = (A * nmu) + b, done per column since
    # the STT scalar must be a single per-partition value
    for j in range(k):
        nc.vector.scalar_tensor_tensor(
            out=outB[:, j:j + 1], in0=outA[:, j:j + 1], scalar=nmu[:, j:j + 1],
            in1=b[:, j:j + 1], op0=OP.mult, op1=OP.add)


# --------------------------------------------------------------------------
# host wrapper
# --------------------------------------------------------------------------

_NC = None


def _get_module():
    global _NC
    if _NC is None:
        _NC = build_module()
    return _NC


def _host_prep(inputs):
    """Build the 8 per-core input maps from full inputs."""
    x = np.asarray(inputs["x"], np.float32)
    haar_weight = np.asarray(inputs["haar_weight"], np.float32)
    conv1_w = np.asarray(inputs["conv1_w"], np.float32)
    conv2_w = np.asarray(inputs["conv2_w"], np.float32)

    # block-diag selector matrices (shared)
    selc = np.zeros((P, CL), np.float32)
    selc[np.arange(P), np.arange(P) % CL] = 1.0
    selb = np.ascontiguousarray(selc.T)

    # conv stationaries: lhsT[(b,i), (b,o)] = W[o,i] within each 16-group
    def blockdiag16(w_oi):  # (16,16) -> (128,128) lhsT
        m = np.zeros((P, P), np.float32)
        for g in range(8):      # 4 b * 2 groups
            m[g * 16:(g + 1) * 16, g * 16:(g + 1) * 16] = w_oi.T
        return m

    w1blk = _to_bf16(blockdiag16(conv1_w[:, :, 0, 0]))
    w2blk = _to_bf16(np.stack([blockdiag16(conv2_w[:, :, dy, dx])
                               for dy in range(3) for dx in range(3)]))

    in_maps = []
    for dd in range(NCORES):
        c0 = CL * dd
        sl = slice(c0, c0 + CL)
        x_core = np.ascontiguousarray(
            x[:, :, sl].transpose(1, 2, 0, 3, 4)).reshape(P, F)
        # block-diag stationaries: lhsT[(b,g,d),(b,g,m)] = Wk[d,m]
        wkblk = np.zeros((4, P, P), np.float32)
        for k in range(4):
            wk = haar_weight[4 * k + dd // 2]
            for g in range(8):
                wkblk[k, g * 16:(g + 1) * 16, g * 16:(g + 1) * 16] = wk
        wk_host = np.ascontiguousarray(wkblk.transpose(1, 0, 2)).reshape(P, 4 * P)
        w2_host = np.ascontiguousarray(w2blk.transpose(1, 0, 2)).reshape(P, 9 * P)

        bnp = np.zeros((CL, 21), np.float32)
        bnp[:, 0] = inputs["bn_fwd_g"][sl]
        bnp[:, 1] = inputs["bn_fwd_g"][C + c0:C + c0 + CL]
        bnp[:, 2] = inputs["bn_fwd_b"][sl]
        bnp[:, 3] = inputs["bn_fwd_b"][C + c0:C + c0 + CL]
        gm = np.asarray(inputs["bn_mul_g"], np.float32).reshape(4, C)[:, sl]
        bm = np.asarray(inputs["bn_mul_b"], np.float32).reshape(4, C)[:, sl]
        bnp[:, 4:8] = gm.T
        bnp[:, 8:12] = bm.T
        bnp[:, 12] = inputs["bn_inv_g"][sl]
        bnp[:, 13] = inputs["bn_c1_g"][sl]
        bnp[:, 14] = inputs["bn_c2_g"][sl]
        bnp[:, 15] = inputs["bn_inv_b"][sl]
        bnp[:, 16] = inputs["bn_c1_b"][sl]
        bnp[:, 17] = inputs["bn_c2_b"][sl]
        bnp[:, 18] = 4e-5   # bn_inv eps (x4: unscaled inverse haar)
        bnp[:, 19] = 1e-5   # bn_c1 eps
        bnp[:, 20] = 1e-5   # bn_c2 eps

        in_maps.append({
            "xin": x_core,
            "w1blk": w1blk,
            "w2blk": w2_host,
            "wkblk": wk_host,
            "selc": selc,
            "selb": selb,
            "bnp": np.ascontiguousarray(bnp),
        })
    return in_maps


def _to_bf16(a):
    return np.asarray(a, dtype=mybir.dt.np(BF16))


def _assemble(results):
    out = np.zeros((T, B, C, H, W), np.float32)
    for dd in range(NCORES):
        oc = np.asarray(results[dd]["out"]).reshape(B, CL, T, H, W)
        out[:, :, CL * dd:CL * (dd + 1)] = oc.transpose(2, 0, 1, 3, 4)
    return out


def kernel(**inputs):
    nc = _get_module()
    in_maps = _host_prep(inputs)
    res = run_bass_kernel_spmd(nc, in_maps, list(range(NCORES)))
    return _assemble(res.results)


if __name__ == "__main__":
    # smoke test with random inputs
    rng = np.random.default_rng(0)
    inputs = {
        "x": rng.standard_normal((T, B, C, H, W)).astype(np.float32),
        "haar_weight": 0.02 * rng.standard_normal((16, 16, 16)).astype(np.float32),
        "conv1_w": 0.1 * rng.standard_normal((16, 16, 1, 1)).astype(np.float32),
        "conv1_b": np.zeros(16, np.float32),
        "conv2_w": 0.05 * rng.standard_normal((16, 16, 3, 3)).astype(np.float32),
        "conv2_b": np.zeros(16, np.float32),
        "bn_fwd_g": np.ones(512, np.float32), "bn_fwd_b": np.zeros(512, np.float32),
        "bn_mul_g": np.ones(1024, np.float32), "bn_mul_b": np.zeros(1024, np.float32),
        "bn_inv_g": np.ones(256, np.float32), "bn_inv_b": np.zeros(256, np.float32),
        "bn_c1_g": np.ones(256, np.float32), "bn_c1_b": np.zeros(256, np.float32),
        "bn_c2_g": np.ones(256, np.float32), "bn_c2_b": np.zeros(256, np.float32),
    }
    out = kernel(**inputs)
    print("out", out.shape, out.dtype, np.abs(out).mean())
